# revision 36
# baseline (speedup 1.0000x reference)
"""Block-local sparse attention (LSG-style) on 8 TRN2 NeuronCores.

Sharding: the 32 (n, h) pairs are split 4-per-core (data/head parallel, no
collectives). Host-side numpy prep re-lays-out the inputs so the device
kernel needs no transposes, all bf16:

  - K-side pack kt[s] = [64, skt | gkt | qt | lkt]: sparse/global/local K^T
    (token-padded with zeros) and Q^T, one DMA per slot.
  - V-side pack vt[s] = [128, gv | sv_pre | lv_pre | sv_suf | lv_suf]:
    V with a ones column appended (col 64), chunked [128, c, 65], every row
    scaled by exp(mask): softmax(QK/8 + m) @ V is computed as
    sum_t exp(s_t) e^{m_t} [V_t, 1], then a divide by the accumulated last
    column - exact for any additive mask, and pad tokens (e^{m}=0) vanish
    from both numerator and denominator, so no mask row and no
    max-subtraction are needed (|QK|/8 is O(5), well within fp32 exp range).
    sv holds 4 phase-shifted copies (chunk-major: [chunk, phase, 65]) so the
    32-token-granular sparse windows always start at partition 0; the
    chunks-0-3 + lv-chunks-0-9 prefix is contiguous, so one ~450KB DMA
    unblocks pairs 0-3 early in slot 0.

Key discovered hardware behavior: matmuls with 64-partition operands run
the PE in a half-array row-group mode at HALF the streaming rate, and
switching modes drains the array (~200ns). All score operands are
therefore zero-padded to 128 partitions (rows 64:128 of kt_t zeroed once
at startup by DVE/ACT, overlapped with the initial loads); every matmul
then streams 1 column / 0.417ns.

The device processes query-block PAIRS: 9 score matmuls per pair into a
3-bank PSUM region [128, 1536] (no matmul output crosses a bank), one
exp(S/8) on ACT (the pacing engine: cols x 0.833ns + one 143ns
PSUM-access bubble), then 12 PV matmuls (N=65) into [q, V|Z] and a
reciprocal-normalize on DVE.  Scores of pair p+2 are interleaved into
PV(p) so pe_s fires mid-iteration and ACT is never input-starved; the
ACT's pp-buffer WAR gate is implied by pe_s (in-order PE), so the exp
carries a single semaphore wait and the ACT chain runs back-to-back.

Raw bass with hand-placed semaphores (walrus: at most one sem wait per
matmul/ACT instruction). Queue assignment: input loads on the GpSimd
queue (3 DMAs per slot), output stores (one merged DMA per pair) on the
Sync queue, so stores never queue behind multi-MB loads; an 8-deep ob
ring rides out store-packet delays behind slot-load bursts in the shared
DMA engines.
"""

from contextlib import ExitStack

import numpy as np

import concourse.bass as bass
import concourse.mybir as mybir
from concourse.bass_utils import run_bass_kernel_spmd

N, H, T, D = 2, 16, 4096, 64
B = 128          # query block
NB = T // B      # 32
G = 64           # global tokens
TSP = T // 4     # sparse tokens (1024)
NH = N * H       # 32
NCORES = 8
SL = NH // NCORES  # 4 heads per core
NP = SL * NB // 2  # 64 block-pairs per core
PPS = NB // 2      # 16 pairs per slot

LKT_W = T + 2 * B            # 4352 padded local tokens
SKT_W = TSP + 320            # 1344 padded sparse tokens
LV_C = LKT_W // 128          # 34 local V chunks
SV_C = 11                    # sparse V chunks per phase

# K pack column offsets: [skt | gkt | qt | lkt]
KO_GKT = SKT_W               # 1344
KO_QT = KO_GKT + 128         # 1472
KO_LKT = KO_QT + T           # 5568
KW = KO_LKT + LKT_W          # 9920

# V pack column offsets: [gv | sv chunks 0-3 | lv chunks 0-9 | rest]
SV_PRE, LV_PRE = 4, 10       # prefix chunk counts (cover pairs hb 0-3)
VO_SV1 = 65
VO_LV1 = VO_SV1 + SV_PRE * 4 * 65        # 1105
VO_SVS = VO_LV1 + LV_PRE * 65            # 1755 = prefix end
VO_LVS = VO_SVS + (SV_C - SV_PRE) * 4 * 65  # 3575
VW = VO_LVS + (LV_C - LV_PRE) * 65       # 5135

F32 = mybir.dt.float32
BF16 = mybir.dt.bfloat16
GE = "sem-ge"

# column layout of the per-pair score/prob tile [128, 1536] (3 PSUM banks;
# regions never cross a 512-col bank boundary)
C_SP1A, C_SP1B = 0, 128
C_SP2A, C_SP2B = 256, 384
C_G = 512        # 256 wide: q of both blocks
C_LOC1 = 768     # 256 wide: local chunk b+1, both blocks
C_LOC0 = 1024    # 128: local chunk b, block A only
C_LOC2 = 1152    # 256 wide: local chunk b+2, both blocks
C_LOC3 = 1408    # 128: local chunk b+3, block B only


def _sv_col(c, q):
    if c < SV_PRE:
        return VO_SV1 + (c * 4 + q) * 65
    return VO_SVS + ((c - SV_PRE) * 4 + q) * 65


def _lv_col(c):
    if c < LV_PRE:
        return VO_LV1 + c * 65
    return VO_LVS + (c - LV_PRE) * 65


def _build_bass():
    nc = bass.Bass("TRN2", num_devices=NCORES, debug=False)

    kt = nc.dram_tensor("kt", [SL, 64, KW], BF16, kind="ExternalInput")
    vt = nc.dram_tensor("vt", [SL, 128, VW], BF16, kind="ExternalInput")
    o = nc.dram_tensor("o", [SL, T, D], F32, kind="ExternalOutput")

    EXP = mybir.ActivationFunctionType.Exp

    with ExitStack() as es:
        ec = es.enter_context
        # double-buffered inputs (slot parity); kt_t rows 64:128 are zeroed
        # once so every matmul contracts over 128 partitions (full PE rate)
        kt_t = [ec(nc.sbuf_tensor(f"kt_t{i}", [128, KW], BF16)) for i in range(2)]
        vt_t = [ec(nc.sbuf_tensor(f"vt_t{i}", [128, VW], BF16)) for i in range(2)]
        # double-buffered per-pair working set (pair parity)
        psS = [ec(nc.psum_tensor(f"psS{i}", [128, 1536], F32)) for i in range(2)]  # 3 banks
        pv = [ec(nc.psum_tensor(f"pv{i}", [128, 512], F32)) for i in range(2)]     # 1 bank
        pp = [ec(nc.sbuf_tensor(f"pp{i}", [128, 1536], BF16)) for i in range(2)]
        rec = [ec(nc.sbuf_tensor(f"rec{i}", [128, 2], F32)) for i in range(2)]
        # 8-deep output ring: slot-load DMA bursts delay store packets by up
        # to ~10us in the shared engines; 8 pairs of slack rides that out.
        OBN = 8
        ob = [ec(nc.sbuf_tensor(f"ob{i}", [128, 128], F32)) for i in range(OBN)]

        diK = [ec(nc.semaphore(f"diK{i}")) for i in range(2)]  # K pack, slot parity
        diV = [ec(nc.semaphore(f"diV{i}")) for i in range(2)]  # V pack, slot parity
        diP = ec(nc.semaphore("diP"))    # slot-0 V prefix (pairs 0-3)
        st = ec(nc.semaphore("st"))      # out stores (+16 per store, FIFO)
        izv = ec(nc.semaphore("izv"))    # kt_t[0] rows 64:128 zeroed (DVE)
        iza = ec(nc.semaphore("iza"))    # kt_t[1] rows 64:128 zeroed (ACT)
        pe_s = ec(nc.semaphore("pe_s"))  # +2 per pair: score matmuls (6, 9) done
        pe_v = ec(nc.semaphore("pe_v"))  # +1 per pair: PV matmuls done
        act = ec(nc.semaphore("act"))    # +1 per pair: exp done
        dve = ec(nc.semaphore("dve"))    # +1 per pair: normalize done
        block = ec(nc.Block(no_gpsimd_drain=True))

        @block.gpsimd
        def _(gpsimd):
            # slot 0: K pack, V prefix (pairs 0-3), V suffix
            gpsimd.dma_start(kt_t[0][0:64, :], kt[0]).then_inc(diK[0], 16)
            gpsimd.dma_start(vt_t[0][:, 0:VO_SVS], vt[0][:, 0:VO_SVS]).then_inc(diP, 16)
            gpsimd.dma_start(vt_t[0][:, VO_SVS:VW], vt[0][:, VO_SVS:VW]).then_inc(diV[0], 16)
            for s, gate in ((1, None), (2, 16), (3, 32)):
                u = s % 2
                d = gpsimd.dma_start(kt_t[u][0:64, :], kt[s])
                if gate is not None:
                    d.wait_op(pe_v, gate, GE)
                d.then_inc(diK[u], 16)
                gpsimd.dma_start(vt_t[u][:], vt[s]).then_inc(diV[u], 16)

        def emit_scores(p, lo=0, hi=9):
            s, hb = divmod(p, PPS)
            b = 2 * hb
            su = s % 2
            if lo == 0 and hb == 0:
                nc.tensor.wait_ge(diK[su], 16 * (s // 2 + 1))
            K = kt_t[su]
            qA = K[:, KO_QT + b * B : KO_QT + (b + 1) * B]
            qB = K[:, KO_QT + (b + 1) * B : KO_QT + (b + 2) * B]
            qAB = K[:, KO_QT + b * B : KO_QT + (b + 2) * B]
            w1a, w2a = 32 * b, 32 * b + 224
            w1b, w2b = w1a + 32, w2a + 32
            u = p % 2
            lo_l = KO_LKT
            mms = (
                (C_SP1A, 128, K[:, w1a : w1a + 128], qA),
                (C_SP1B, 128, K[:, w1b : w1b + 128], qB),
                (C_SP2A, 128, K[:, w2a : w2a + 128], qA),
                (C_SP2B, 128, K[:, w2b : w2b + 128], qB),
                (C_G, 256, K[:, KO_GKT : KO_GKT + 128], qAB),
                (C_LOC1, 256, K[:, lo_l + (b + 1) * B : lo_l + (b + 2) * B], qAB),
                (C_LOC0, 128, K[:, lo_l + b * B : lo_l + (b + 1) * B], qA),
                (C_LOC2, 256, K[:, lo_l + (b + 2) * B : lo_l + (b + 3) * B], qAB),
                (C_LOC3, 128, K[:, lo_l + (b + 3) * B : lo_l + (b + 4) * B], qB),
            )
            for kk in range(lo, hi):
                col, w, lhsT, rhs = mms[kk]
                mm = nc.tensor.matmul(
                    psS[u][:, col : col + w],
                    lhsT, rhs,
                    start=True, stop=True,
                )
                if kk == 5 or kk == 8:
                    mm.then_inc(pe_s, 1)

        def pv_mms(p):
            s, hb = divmod(p, PPS)
            b = 2 * hb
            u = p % 2
            su = s % 2
            V = vt_t[su]
            sp = []
            for blk in range(2):
                bb = b + blk
                w1, w2 = 32 * bb, 32 * bb + 224
                c1, r1 = divmod(w1, 128)
                c2, r2 = divmod(w2, 128)
                sp.append((_sv_col(c1, r1 // 32), _sv_col(c2, r2 // 32)))
            lv = [_lv_col(b + k) for k in range(4)]
            outA = pv[u][:, 0:65]
            outB = pv[u][:, 128:193]
            # Sequential accumulation groups (A fully, then B): a start=True
            # marks the surrounding 2KB PSUM zero-region pending-zero, so two
            # interleaved in-flight groups in one bank corrupt each other.
            # (out, pp col, rhs, start, stop)
            return (
                (outA, C_SP1A, V[:, sp[0][0] : sp[0][0] + 65], True, False),
                (outA, C_SP2A, V[:, sp[0][1] : sp[0][1] + 65], False, False),
                (outA, C_G, V[:, 0:65], False, False),
                (outA, C_LOC1, V[:, lv[1] : lv[1] + 65], False, False),
                (outA, C_LOC0, V[:, lv[0] : lv[0] + 65], False, False),
                (outA, C_LOC2, V[:, lv[2] : lv[2] + 65], False, True),
                (outB, C_SP1B, V[:, sp[1][0] : sp[1][0] + 65], True, False),
                (outB, C_SP2B, V[:, sp[1][1] : sp[1][1] + 65], False, False),
                (outB, C_G + 128, V[:, 0:65], False, False),
                (outB, C_LOC1 + 128, V[:, lv[1] : lv[1] + 65], False, False),
                (outB, C_LOC2 + 128, V[:, lv[2] : lv[2] + 65], False, False),
                (outB, C_LOC3, V[:, lv[3] : lv[3] + 65], False, True),
            )

        def emit_pv_range(p, mms, lo, hi):
            u = p % 2
            for kk in range(lo, hi):
                out, col, rhs, st_, sp_ = mms[kk]
                mm = nc.tensor.matmul(
                    out, pp[u][:, col : col + 128], rhs,
                    start=st_, stop=sp_, skip_group_check=True,
                )
                if kk == 11:
                    mm.then_inc(pe_v, 1)

        @block.tensor
        def _(tensor):
            tensor.wait_ge(izv, 1)
            tensor.wait_ge(iza, 1)
            tensor.wait_ge(diK[0], 16)
            emit_scores(0)
            emit_scores(1)
            for p in range(NP):
                s, hb = divmod(p, PPS)
                su = s % 2
                if p >= 2:
                    tensor.wait_ge(dve, p - 1)  # pv[u] free
                if s == 0:
                    if hb == 0:
                        tensor.wait_ge(diP, 16)      # V prefix: pairs 0-3
                    elif hb == 4:
                        tensor.wait_ge(diV[0], 16)   # slot-0 V suffix
                elif hb == 0:
                    tensor.wait_ge(diV[su], 16 * (s // 2 + 1))
                mms = pv_mms(p)
                # everything below needs only exp(p) done; scores(p+2) are
                # interleaved so pe_s fires mid-iteration, keeping ACT fed
                tensor.wait_ge(act, p + 1)
                emit_pv_range(p, mms, 0, 4)   # A: sp1 sp2 G loc1
                if p + 2 < NP:
                    emit_scores(p + 2, 0, 6)  # sp x4, G, LOC1
                emit_pv_range(p, mms, 4, 12)  # A: loc0 loc2; B: all
                if p + 2 < NP:
                    emit_scores(p + 2, 6, 9)  # LOC0 LOC2 LOC3

        @block.scalar
        def _(scalar):
            nc.scalar.memzero(kt_t[1][64:128, :]).then_inc(iza, 1)
            # one exp per pair: ACT is the pacer and each ACTIVATE pays a
            # 143ns PSUM-access bubble, so a single op is cheapest.  No pe_v
            # wait needed: pe_s >= 2p+2 means scores(p) mm8 is done, which
            # the in-order PE completed after PV(p-2)'s last matmul, so
            # pp[u] is already free.
            for p in range(NP):
                u = p % 2
                nc.scalar.activation(
                    pp[u][:, 0:1536], psS[u][:, 0:1536], EXP, scale=0.125
                ).wait_op(pe_s, 2 * p + 2, GE).then_inc(act, 1)

        @block.vector
        def _(vector):
            nc.vector.memzero(kt_t[0][64:128, :]).then_inc(izv, 1)
            for p in range(NP):
                u = p % 2
                w = p % OBN
                if p >= OBN:
                    vector.wait_ge(st, 16 * (p - OBN + 1))  # ob[w] stored
                nc.vector.reciprocal(rec[u][:, 0:1], pv[u][:, 64:65]).wait_op(
                    pe_v, p + 1, GE
                )
                nc.vector.reciprocal(rec[u][:, 1:2], pv[u][:, 192:193])
                nc.vector.drain()  # DVE pipeline RAW: rec written, read next
                nc.vector.tensor_mul(
                    ob[w][:, 0:64], pv[u][:, 0:64],
                    rec[u][:, 0:1].broadcast_to([128, 64]),
                )
                nc.vector.tensor_mul(
                    ob[w][:, 64:128], pv[u][:, 128:192],
                    rec[u][:, 1:2].broadcast_to([128, 64]),
                ).then_inc(dve, 1)

        @block.sync
        def _(sync):
            for p in range(NP):
                s, hb = divmod(p, PPS)
                b = 2 * hb
                dst = o[s, b * B : (b + 2) * B, :].rearrange(
                    "(blk q) d -> q blk d", blk=2
                )
                src = ob[p % OBN][:, 0:128].rearrange("q (blk d) -> q blk d", blk=2)
                sync.dma_start(dst, src).wait_op(dve, p + 1, GE).then_inc(st, 16)
            sync.wait_ge(st, 16 * NP)

    return nc


def _prepare(inputs):
    import ml_dtypes

    bf = ml_dtypes.bfloat16
    f = np.float32
    q = np.asarray(inputs["query_layer"], f).reshape(NH, T, D)
    k = np.asarray(inputs["key_layer"], f).reshape(NH, T, D)
    v = np.asarray(inputs["value_layer"], f).reshape(NH, T, D)
    sk = np.asarray(inputs["sparse_key"], f).reshape(NH, TSP, D)
    svv = np.asarray(inputs["sparse_value"], f).reshape(NH, TSP, D)
    gk = np.asarray(inputs["global_key"], f).reshape(NH, G, D)
    gvv = np.asarray(inputs["global_value"], f).reshape(NH, G, D)
    am = np.repeat(np.asarray(inputs["attention_mask"], f)[:, 0, 0, :], H, 0)
    sm = np.repeat(np.asarray(inputs["sparse_mask"], f)[:, 0, 0, :], H, 0)
    gm = np.repeat(np.asarray(inputs["global_mask"], f)[:, 0, 0, :], H, 0)

    # K-side pack [skt | gkt | qt | lkt], all K^T / Q^T as [64, tokens]
    kpack = np.zeros((NH, 64, KW), f)
    kpack[:, :, 160 : 160 + TSP] = sk.transpose(0, 2, 1)
    kpack[:, :, KO_GKT : KO_GKT + G] = gk.transpose(0, 2, 1)
    kpack[:, :, KO_QT : KO_QT + T] = q.transpose(0, 2, 1)
    kpack[:, :, KO_LKT + B : KO_LKT + B + T] = k.transpose(0, 2, 1)
    kpack = kpack.astype(bf)

    # V_aug rows scaled by exp(mask); pad rows are all-zero
    em_l = np.zeros((NH, LKT_W), f)
    em_l[:, B : B + T] = np.exp(am)
    lvp = np.zeros((NH, LKT_W, 65), f)
    lvp[:, B : B + T, :64] = v
    lvp[:, :, 64] = 1.0
    lvp *= em_l[:, :, None]
    lvp = np.ascontiguousarray(
        lvp.reshape(NH, LV_C, 128, 65).transpose(0, 2, 1, 3)
    ).reshape(NH, 128, LV_C * 65)

    SVP_W = 96 + SV_C * 128
    em_s = np.zeros((NH, SVP_W), f)
    em_s[:, 160 : 160 + TSP] = np.exp(sm)
    sv_pad = np.zeros((NH, SVP_W, 65), f)
    sv_pad[:, 160 : 160 + TSP, :64] = svv
    sv_pad[:, :, 64] = 1.0
    sv_pad *= em_s[:, :, None]
    svp = np.empty((NH, 4, 128, SV_C, 65), f)
    for ph in range(4):
        svp[:, ph] = (
            sv_pad[:, 32 * ph : 32 * ph + SV_C * 128]
            .reshape(NH, SV_C, 128, 65)
            .transpose(0, 2, 1, 3)
        )
    # chunk-major [chunk, phase, 65] so the chunks 0-3 prefix is contiguous
    svp = np.ascontiguousarray(svp.transpose(0, 2, 3, 1, 4))  # [NH,128,SV_C,4,65]

    gvp = np.zeros((NH, 128, 65), f)
    gvp[:, :G, :64] = gvv
    gvp[:, :G, 64] = 1.0
    gvp[:, :G] *= np.exp(gm)[:, :, None]

    vpack = np.concatenate(
        [
            gvp,
            svp[:, :, :SV_PRE].reshape(NH, 128, SV_PRE * 4 * 65),
            lvp[:, :, : LV_PRE * 65],
            svp[:, :, SV_PRE:].reshape(NH, 128, (SV_C - SV_PRE) * 4 * 65),
            lvp[:, :, LV_PRE * 65 :],
        ],
        axis=2,
    ).astype(bf)

    return [
        {
            "kt": kpack[c * SL : (c + 1) * SL],
            "vt": vpack[c * SL : (c + 1) * SL],
        }
        for c in range(NCORES)
    ]


_NC_CACHE = {}
LAST_RESULTS = None


def kernel(**inputs):
    global LAST_RESULTS
    if "nc" not in _NC_CACHE:
        _NC_CACHE["nc"] = _build_bass()
    nc = _NC_CACHE["nc"]
    in_maps = _prepare(inputs)
    res = run_bass_kernel_spmd(nc, in_maps, core_ids=list(range(NCORES)))
    LAST_RESULTS = res
    out = np.empty((NH, T, D), np.float32)
    for c in range(NCORES):
        out[c * SL : (c + 1) * SL] = res.results[c]["o"]
    return out.reshape(N, H, T, D)


# revision 40
# speedup vs baseline: 1.0088x; 1.0088x over previous
"""Block-local sparse attention (LSG-style) on 8 TRN2 NeuronCores.

Sharding: the 32 (n, h) pairs are split 4-per-core (data/head parallel, no
collectives). Host-side numpy prep re-lays-out the inputs so the device
kernel needs no transposes, all bf16:

  - K-side pack kt[s] = [64, skt | gkt | qt | lkt]: sparse/global/local K^T
    (token-padded with zeros) and Q^T, one DMA per slot.
  - V-side pack vt[s] = [128, gv | sv_pre | lv_pre | sv_suf | lv_suf]:
    V with a ones column appended (col 64), chunked [128, c, 65], every row
    scaled by exp(mask): softmax(QK/8 + m) @ V is computed as
    sum_t exp(s_t) e^{m_t} [V_t, 1], then a divide by the accumulated last
    column - exact for any additive mask, and pad tokens (e^{m}=0) vanish
    from both numerator and denominator, so no mask row and no
    max-subtraction are needed (|QK|/8 is O(5), well within fp32 exp range).
    sv holds 4 phase-shifted copies (chunk-major: [chunk, phase, 65]) so the
    32-token-granular sparse windows always start at partition 0; the
    chunks-0-3 + lv-chunks-0-9 prefix is contiguous, so one ~450KB DMA
    unblocks pairs 0-3 early in slot 0.

Key discovered hardware behavior: matmuls with 64-partition operands run
the PE in a half-array row-group mode at HALF the streaming rate, and
switching modes drains the array (~200ns). All score operands are
therefore zero-padded to 128 partitions (rows 64:128 of kt_t zeroed once
at startup by DVE/ACT, overlapped with the initial loads); every matmul
then streams 1 column / 0.417ns.

The device processes query-block PAIRS: 9 score matmuls per pair into a
3-bank PSUM region [128, 1536] (no matmul output crosses a bank), one
exp(S/8) on ACT (the pacing engine: cols x 0.833ns + one 143ns
PSUM-access bubble), then 12 PV matmuls (N=65) into [q, V|Z] and a
reciprocal-normalize on DVE.  Scores of pair p+2 are interleaved into
PV(p) so pe_s fires mid-iteration and ACT is never input-starved; the
ACT's pp-buffer WAR gate is implied by pe_s (in-order PE), so the exp
carries a single semaphore wait and the ACT chain runs back-to-back.

Raw bass with hand-placed semaphores (walrus: at most one sem wait per
matmul/ACT instruction). Queue assignment: input loads on the GpSimd
queue (3 DMAs per slot), output stores (one merged DMA per pair) on the
Sync queue, so stores never queue behind multi-MB loads; an 8-deep ob
ring rides out store-packet delays behind slot-load bursts in the shared
DMA engines.
"""

from contextlib import ExitStack

import numpy as np

import concourse.bass as bass
import concourse.mybir as mybir
from concourse.bass_utils import run_bass_kernel_spmd

N, H, T, D = 2, 16, 4096, 64
B = 128          # query block
NB = T // B      # 32
G = 64           # global tokens
TSP = T // 4     # sparse tokens (1024)
NH = N * H       # 32
NCORES = 8
SL = NH // NCORES  # 4 heads per core
NP = SL * NB // 2  # 64 block-pairs per core
PPS = NB // 2      # 16 pairs per slot

LKT_W = T + 2 * B            # 4352 padded local tokens
SKT_W = TSP + 320            # 1344 padded sparse tokens
LV_C = LKT_W // 128          # 34 local V chunks
SV_C = 11                    # sparse V chunks per phase

# K pack column offsets: [skt | gkt | qt | lkt]
KO_GKT = SKT_W               # 1344
KO_QT = KO_GKT + 128         # 1472
KO_LKT = KO_QT + T           # 5568
KW = KO_LKT + LKT_W          # 9920

# V pack column offsets: [gv | sv chunks 0-3 | lv chunks 0-9 | rest]
SV_PRE, LV_PRE = 4, 10       # prefix chunk counts (cover pairs hb 0-3)
VO_SV1 = 65
VO_LV1 = VO_SV1 + SV_PRE * 4 * 65        # 1105
VO_SVS = VO_LV1 + LV_PRE * 65            # 1755 = prefix end
VO_LVS = VO_SVS + (SV_C - SV_PRE) * 4 * 65  # 3575
VW = VO_LVS + (LV_C - LV_PRE) * 65       # 5135

F32 = mybir.dt.float32
BF16 = mybir.dt.bfloat16
GE = "sem-ge"

# column layout of the per-pair score/prob tile [128, 1536] (3 PSUM banks;
# regions never cross a 512-col bank boundary)
C_SP1A, C_SP1B = 0, 128
C_SP2A, C_SP2B = 256, 384
C_G = 512        # 256 wide: q of both blocks
C_LOC1 = 768     # 256 wide: local chunk b+1, both blocks
C_LOC0 = 1024    # 128: local chunk b, block A only
C_LOC2 = 1152    # 256 wide: local chunk b+2, both blocks
C_LOC3 = 1408    # 128: local chunk b+3, block B only


def _sv_col(c, q):
    if c < SV_PRE:
        return VO_SV1 + (c * 4 + q) * 65
    return VO_SVS + ((c - SV_PRE) * 4 + q) * 65


def _lv_col(c):
    if c < LV_PRE:
        return VO_LV1 + c * 65
    return VO_LVS + (c - LV_PRE) * 65


def _build_bass():
    nc = bass.Bass("TRN2", num_devices=NCORES, debug=False)

    kt = nc.dram_tensor("kt", [SL, 64, KW], BF16, kind="ExternalInput")
    vt = nc.dram_tensor("vt", [SL, 128, VW], BF16, kind="ExternalInput")
    o = nc.dram_tensor("o", [SL, T, D], F32, kind="ExternalOutput")

    EXP = mybir.ActivationFunctionType.Exp

    with ExitStack() as es:
        ec = es.enter_context
        # double-buffered inputs (slot parity); kt_t rows 64:128 are zeroed
        # once so every matmul contracts over 128 partitions (full PE rate)
        kt_t = [ec(nc.sbuf_tensor(f"kt_t{i}", [128, KW], BF16)) for i in range(2)]
        vt_t = [ec(nc.sbuf_tensor(f"vt_t{i}", [128, VW], BF16)) for i in range(2)]
        # double-buffered per-pair working set (pair parity)
        psS = [ec(nc.psum_tensor(f"psS{i}", [128, 1536], F32)) for i in range(2)]  # 3 banks
        pv = [ec(nc.psum_tensor(f"pv{i}", [128, 512], F32)) for i in range(2)]     # 1 bank
        pp = [ec(nc.sbuf_tensor(f"pp{i}", [128, 1536], BF16)) for i in range(2)]
        rec = [ec(nc.sbuf_tensor(f"rec{i}", [128, 2], F32)) for i in range(2)]
        # 16-deep output ring: slot-load DMA bursts delay store packets by
        # up to ~12us in the shared engines; 16 pairs of slack rides it out.
        OBN = 16
        ob = [ec(nc.sbuf_tensor(f"ob{i}", [128, 128], F32)) for i in range(OBN)]

        diK = [ec(nc.semaphore(f"diK{i}")) for i in range(2)]  # K pack, slot parity
        diV = [ec(nc.semaphore(f"diV{i}")) for i in range(2)]  # V pack, slot parity
        diP = ec(nc.semaphore("diP"))    # slot-0 V prefix (pairs 0-3)
        st = ec(nc.semaphore("st"))      # out stores (+16 per store, FIFO)
        izv = ec(nc.semaphore("izv"))    # kt_t[0] rows 64:128 zeroed (DVE)
        iza = ec(nc.semaphore("iza"))    # kt_t[1] rows 64:128 zeroed (ACT)
        pe_s = ec(nc.semaphore("pe_s"))  # +2 per pair: score matmuls (6, 9) done
        pe_v = ec(nc.semaphore("pe_v"))  # +1 per pair: PV matmuls done
        act = ec(nc.semaphore("act"))    # +1 per pair: exp done
        dve = ec(nc.semaphore("dve"))    # +1 per pair: normalize done
        block = ec(nc.Block(no_gpsimd_drain=True))

        @block.gpsimd
        def _(gpsimd):
            # slot 0: K pack, V prefix (pairs 0-3), V suffix
            gpsimd.dma_start(kt_t[0][0:64, :], kt[0]).then_inc(diK[0], 16)
            gpsimd.dma_start(vt_t[0][:, 0:VO_SVS], vt[0][:, 0:VO_SVS]).then_inc(diP, 16)
            gpsimd.dma_start(vt_t[0][:, VO_SVS:VW], vt[0][:, VO_SVS:VW]).then_inc(diV[0], 16)
            # slots 1-3 split in ~650KB pieces so store packets can slip
            # between load blobs in the DMA-engine FIFOs
            KH, VH = KW // 2, VW // 2
            for s, gate in ((1, None), (2, 16), (3, 32)):
                u = s % 2
                d = gpsimd.dma_start(kt_t[u][0:64, 0:KH], kt[s][:, 0:KH])
                if gate is not None:
                    d.wait_op(pe_v, gate, GE)
                d.then_inc(diK[u], 16)
                gpsimd.dma_start(kt_t[u][0:64, KH:KW], kt[s][:, KH:KW]).then_inc(diK[u], 16)
                gpsimd.dma_start(vt_t[u][:, 0:VH], vt[s][:, 0:VH]).then_inc(diV[u], 16)
                gpsimd.dma_start(vt_t[u][:, VH:VW], vt[s][:, VH:VW]).then_inc(diV[u], 16)

        def emit_scores(p, lo=0, hi=9):
            s, hb = divmod(p, PPS)
            b = 2 * hb
            su = s % 2
            if lo == 0 and hb == 0:
                # slot 0 loads K in 1 piece (16), slots 1-3 in 2 pieces (32)
                nc.tensor.wait_ge(diK[su], 16 + 32 * (s // 2) if su == 0
                                  else 32 * (s // 2 + 1))
            K = kt_t[su]
            qA = K[:, KO_QT + b * B : KO_QT + (b + 1) * B]
            qB = K[:, KO_QT + (b + 1) * B : KO_QT + (b + 2) * B]
            qAB = K[:, KO_QT + b * B : KO_QT + (b + 2) * B]
            w1a, w2a = 32 * b, 32 * b + 224
            w1b, w2b = w1a + 32, w2a + 32
            u = p % 2
            lo_l = KO_LKT
            mms = (
                (C_SP1A, 128, K[:, w1a : w1a + 128], qA),
                (C_SP1B, 128, K[:, w1b : w1b + 128], qB),
                (C_SP2A, 128, K[:, w2a : w2a + 128], qA),
                (C_SP2B, 128, K[:, w2b : w2b + 128], qB),
                (C_G, 256, K[:, KO_GKT : KO_GKT + 128], qAB),
                (C_LOC1, 256, K[:, lo_l + (b + 1) * B : lo_l + (b + 2) * B], qAB),
                (C_LOC0, 128, K[:, lo_l + b * B : lo_l + (b + 1) * B], qA),
                (C_LOC2, 256, K[:, lo_l + (b + 2) * B : lo_l + (b + 3) * B], qAB),
                (C_LOC3, 128, K[:, lo_l + (b + 3) * B : lo_l + (b + 4) * B], qB),
            )
            for kk in range(lo, hi):
                col, w, lhsT, rhs = mms[kk]
                mm = nc.tensor.matmul(
                    psS[u][:, col : col + w],
                    lhsT, rhs,
                    start=True, stop=True,
                )
                if kk == 5 or kk == 8:
                    mm.then_inc(pe_s, 1)

        def pv_mms(p):
            s, hb = divmod(p, PPS)
            b = 2 * hb
            u = p % 2
            su = s % 2
            V = vt_t[su]
            sp = []
            for blk in range(2):
                bb = b + blk
                w1, w2 = 32 * bb, 32 * bb + 224
                c1, r1 = divmod(w1, 128)
                c2, r2 = divmod(w2, 128)
                sp.append((_sv_col(c1, r1 // 32), _sv_col(c2, r2 // 32)))
            lv = [_lv_col(b + k) for k in range(4)]
            outA = pv[u][:, 0:65]
            outB = pv[u][:, 128:193]
            # Sequential accumulation groups (A fully, then B): a start=True
            # marks the surrounding 2KB PSUM zero-region pending-zero, so two
            # interleaved in-flight groups in one bank corrupt each other.
            # (out, pp col, rhs, start, stop)
            return (
                (outA, C_SP1A, V[:, sp[0][0] : sp[0][0] + 65], True, False),
                (outA, C_SP2A, V[:, sp[0][1] : sp[0][1] + 65], False, False),
                (outA, C_G, V[:, 0:65], False, False),
                (outA, C_LOC1, V[:, lv[1] : lv[1] + 65], False, False),
                (outA, C_LOC0, V[:, lv[0] : lv[0] + 65], False, False),
                (outA, C_LOC2, V[:, lv[2] : lv[2] + 65], False, True),
                (outB, C_SP1B, V[:, sp[1][0] : sp[1][0] + 65], True, False),
                (outB, C_SP2B, V[:, sp[1][1] : sp[1][1] + 65], False, False),
                (outB, C_G + 128, V[:, 0:65], False, False),
                (outB, C_LOC1 + 128, V[:, lv[1] : lv[1] + 65], False, False),
                (outB, C_LOC2 + 128, V[:, lv[2] : lv[2] + 65], False, False),
                (outB, C_LOC3, V[:, lv[3] : lv[3] + 65], False, True),
            )

        def emit_pv_range(p, mms, lo, hi):
            u = p % 2
            for kk in range(lo, hi):
                out, col, rhs, st_, sp_ = mms[kk]
                mm = nc.tensor.matmul(
                    out, pp[u][:, col : col + 128], rhs,
                    start=st_, stop=sp_, skip_group_check=True,
                )
                if kk == 11:
                    mm.then_inc(pe_v, 1)

        @block.tensor
        def _(tensor):
            tensor.wait_ge(izv, 1)
            tensor.wait_ge(iza, 1)
            tensor.wait_ge(diK[0], 16)
            emit_scores(0)
            emit_scores(1)
            for p in range(NP):
                s, hb = divmod(p, PPS)
                su = s % 2
                if p >= 2:
                    tensor.wait_ge(dve, p - 1)  # pv[u] free
                if s == 0:
                    if hb == 0:
                        tensor.wait_ge(diP, 16)      # V prefix: pairs 0-3
                    elif hb == 4:
                        tensor.wait_ge(diV[0], 16)   # slot-0 V suffix
                elif hb == 0:
                    # slot 0 V is 1 diV inc (16), slots 1-3 are 2 (32)
                    tensor.wait_ge(diV[su], 16 + 32 * (s // 2) if su == 0
                                   else 32 * (s // 2 + 1))
                mms = pv_mms(p)
                # everything below needs only exp(p) done; scores(p+2) are
                # interleaved so pe_s fires mid-iteration, keeping ACT fed
                tensor.wait_ge(act, p + 1)
                emit_pv_range(p, mms, 0, 4)   # A: sp1 sp2 G loc1
                if p + 2 < NP:
                    emit_scores(p + 2, 0, 6)  # sp x4, G, LOC1
                emit_pv_range(p, mms, 4, 12)  # A: loc0 loc2; B: all
                if p + 2 < NP:
                    emit_scores(p + 2, 6, 9)  # LOC0 LOC2 LOC3

        @block.scalar
        def _(scalar):
            nc.scalar.memzero(kt_t[1][64:128, :]).then_inc(iza, 1)
            # one exp per pair: ACT is the pacer and each ACTIVATE pays a
            # 143ns PSUM-access bubble, so a single op is cheapest.  No pe_v
            # wait needed: pe_s >= 2p+2 means scores(p) mm8 is done, which
            # the in-order PE completed after PV(p-2)'s last matmul, so
            # pp[u] is already free.
            for p in range(NP):
                u = p % 2
                nc.scalar.activation(
                    pp[u][:, 0:1536], psS[u][:, 0:1536], EXP, scale=0.125
                ).wait_op(pe_s, 2 * p + 2, GE).then_inc(act, 1)

        @block.vector
        def _(vector):
            nc.vector.memzero(kt_t[0][64:128, :]).then_inc(izv, 1)
            for p in range(NP):
                u = p % 2
                w = p % OBN
                if p >= OBN:
                    vector.wait_ge(st, 16 * (p - OBN + 1))  # ob[w] stored
                nc.vector.reciprocal(rec[u][:, 0:1], pv[u][:, 64:65]).wait_op(
                    pe_v, p + 1, GE
                )
                nc.vector.reciprocal(rec[u][:, 1:2], pv[u][:, 192:193])
                nc.vector.drain()  # DVE pipeline RAW: rec written, read next
                nc.vector.tensor_mul(
                    ob[w][:, 0:64], pv[u][:, 0:64],
                    rec[u][:, 0:1].broadcast_to([128, 64]),
                )
                nc.vector.tensor_mul(
                    ob[w][:, 64:128], pv[u][:, 128:192],
                    rec[u][:, 1:2].broadcast_to([128, 64]),
                ).then_inc(dve, 1)

        @block.sync
        def _(sync):
            for p in range(NP):
                s, hb = divmod(p, PPS)
                b = 2 * hb
                dst = o[s, b * B : (b + 2) * B, :].rearrange(
                    "(blk q) d -> q blk d", blk=2
                )
                src = ob[p % OBN][:, 0:128].rearrange("q (blk d) -> q blk d", blk=2)
                sync.dma_start(dst, src).wait_op(dve, p + 1, GE).then_inc(st, 16)
            sync.wait_ge(st, 16 * NP)

    return nc


def _prepare(inputs):
    import ml_dtypes

    bf = ml_dtypes.bfloat16
    f = np.float32
    q = np.asarray(inputs["query_layer"], f).reshape(NH, T, D)
    k = np.asarray(inputs["key_layer"], f).reshape(NH, T, D)
    v = np.asarray(inputs["value_layer"], f).reshape(NH, T, D)
    sk = np.asarray(inputs["sparse_key"], f).reshape(NH, TSP, D)
    svv = np.asarray(inputs["sparse_value"], f).reshape(NH, TSP, D)
    gk = np.asarray(inputs["global_key"], f).reshape(NH, G, D)
    gvv = np.asarray(inputs["global_value"], f).reshape(NH, G, D)
    am = np.repeat(np.asarray(inputs["attention_mask"], f)[:, 0, 0, :], H, 0)
    sm = np.repeat(np.asarray(inputs["sparse_mask"], f)[:, 0, 0, :], H, 0)
    gm = np.repeat(np.asarray(inputs["global_mask"], f)[:, 0, 0, :], H, 0)

    # K-side pack [skt | gkt | qt | lkt], all K^T / Q^T as [64, tokens]
    kpack = np.zeros((NH, 64, KW), f)
    kpack[:, :, 160 : 160 + TSP] = sk.transpose(0, 2, 1)
    kpack[:, :, KO_GKT : KO_GKT + G] = gk.transpose(0, 2, 1)
    kpack[:, :, KO_QT : KO_QT + T] = q.transpose(0, 2, 1)
    kpack[:, :, KO_LKT + B : KO_LKT + B + T] = k.transpose(0, 2, 1)
    kpack = kpack.astype(bf)

    # V_aug rows scaled by exp(mask); pad rows are all-zero
    em_l = np.zeros((NH, LKT_W), f)
    em_l[:, B : B + T] = np.exp(am)
    lvp = np.zeros((NH, LKT_W, 65), f)
    lvp[:, B : B + T, :64] = v
    lvp[:, :, 64] = 1.0
    lvp *= em_l[:, :, None]
    lvp = np.ascontiguousarray(
        lvp.reshape(NH, LV_C, 128, 65).transpose(0, 2, 1, 3)
    ).reshape(NH, 128, LV_C * 65)

    SVP_W = 96 + SV_C * 128
    em_s = np.zeros((NH, SVP_W), f)
    em_s[:, 160 : 160 + TSP] = np.exp(sm)
    sv_pad = np.zeros((NH, SVP_W, 65), f)
    sv_pad[:, 160 : 160 + TSP, :64] = svv
    sv_pad[:, :, 64] = 1.0
    sv_pad *= em_s[:, :, None]
    svp = np.empty((NH, 4, 128, SV_C, 65), f)
    for ph in range(4):
        svp[:, ph] = (
            sv_pad[:, 32 * ph : 32 * ph + SV_C * 128]
            .reshape(NH, SV_C, 128, 65)
            .transpose(0, 2, 1, 3)
        )
    # chunk-major [chunk, phase, 65] so the chunks 0-3 prefix is contiguous
    svp = np.ascontiguousarray(svp.transpose(0, 2, 3, 1, 4))  # [NH,128,SV_C,4,65]

    gvp = np.zeros((NH, 128, 65), f)
    gvp[:, :G, :64] = gvv
    gvp[:, :G, 64] = 1.0
    gvp[:, :G] *= np.exp(gm)[:, :, None]

    vpack = np.concatenate(
        [
            gvp,
            svp[:, :, :SV_PRE].reshape(NH, 128, SV_PRE * 4 * 65),
            lvp[:, :, : LV_PRE * 65],
            svp[:, :, SV_PRE:].reshape(NH, 128, (SV_C - SV_PRE) * 4 * 65),
            lvp[:, :, LV_PRE * 65 :],
        ],
        axis=2,
    ).astype(bf)

    return [
        {
            "kt": kpack[c * SL : (c + 1) * SL],
            "vt": vpack[c * SL : (c + 1) * SL],
        }
        for c in range(NCORES)
    ]


_NC_CACHE = {}
LAST_RESULTS = None


def kernel(**inputs):
    global LAST_RESULTS
    if "nc" not in _NC_CACHE:
        _NC_CACHE["nc"] = _build_bass()
    nc = _NC_CACHE["nc"]
    in_maps = _prepare(inputs)
    res = run_bass_kernel_spmd(nc, in_maps, core_ids=list(range(NCORES)))
    LAST_RESULTS = res
    out = np.empty((NH, T, D), np.float32)
    for c in range(NCORES):
        out[c * SL : (c + 1) * SL] = res.results[c]["o"]
    return out.reshape(N, H, T, D)


# revision 41
# speedup vs baseline: 1.0599x; 1.0507x over previous
"""Block-local sparse attention (LSG-style) on 8 TRN2 NeuronCores.

Sharding: the 32 (n, h) pairs are split 4-per-core (data/head parallel, no
collectives). Host-side numpy prep re-lays-out the inputs so the device
kernel needs no transposes, all bf16:

  - K-side pack kt[s] = [64, skt | gkt | qt | lkt]: sparse/global/local K^T
    (token-padded with zeros) and Q^T, one DMA per slot.
  - V-side pack vt[s] = [128, gv | sv_pre | lv_pre | sv_suf | lv_suf]:
    V with a ones column appended (col 64), chunked [128, c, 65], every row
    scaled by exp(mask): softmax(QK/8 + m) @ V is computed as
    sum_t exp(s_t) e^{m_t} [V_t, 1], then a divide by the accumulated last
    column - exact for any additive mask, and pad tokens (e^{m}=0) vanish
    from both numerator and denominator, so no mask row and no
    max-subtraction are needed (|QK|/8 is O(5), well within fp32 exp range).
    sv holds 4 phase-shifted copies (chunk-major: [chunk, phase, 65]) so the
    32-token-granular sparse windows always start at partition 0; the
    chunks-0-3 + lv-chunks-0-9 prefix is contiguous, so one ~450KB DMA
    unblocks pairs 0-3 early in slot 0.

Key discovered hardware behavior: matmuls with 64-partition operands run
the PE in a half-array row-group mode at HALF the streaming rate, and
switching modes drains the array (~200ns). All score operands are
therefore zero-padded to 128 partitions (rows 64:128 of kt_t zeroed once
at startup by DVE/ACT, overlapped with the initial loads); every matmul
then streams 1 column / 0.417ns.

The device processes query-block PAIRS: 9 score matmuls per pair into a
3-bank PSUM region [128, 1536] (no matmul output crosses a bank), one
exp(S/8) on ACT (the pacing engine: cols x 0.833ns + one 143ns
PSUM-access bubble), then 12 PV matmuls (N=65) into [q, V|Z] and a
reciprocal-normalize on DVE.  Scores of pair p+2 are interleaved into
PV(p) so pe_s fires mid-iteration and ACT is never input-starved; the
ACT's pp-buffer WAR gate is implied by pe_s (in-order PE), so the exp
carries a single semaphore wait and the ACT chain runs back-to-back.

Raw bass with hand-placed semaphores (walrus: at most one sem wait per
matmul/ACT instruction). Queue assignment: input loads on the GpSimd
queue (3 DMAs per slot), output stores (one merged DMA per pair) on the
Sync queue, so stores never queue behind multi-MB loads; an 8-deep ob
ring rides out store-packet delays behind slot-load bursts in the shared
DMA engines.
"""

from contextlib import ExitStack

import numpy as np

import concourse.bass as bass
import concourse.mybir as mybir
from concourse.bass_utils import run_bass_kernel_spmd

N, H, T, D = 2, 16, 4096, 64
B = 128          # query block
NB = T // B      # 32
G = 64           # global tokens
TSP = T // 4     # sparse tokens (1024)
NH = N * H       # 32
NCORES = 8
SL = NH // NCORES  # 4 heads per core
NP = SL * NB // 2  # 64 block-pairs per core
PPS = NB // 2      # 16 pairs per slot

LKT_W = T + 2 * B            # 4352 padded local tokens
SKT_W = TSP + 320            # 1344 padded sparse tokens
LV_C = LKT_W // 128          # 34 local V chunks
SV_C = 11                    # sparse V chunks per phase

# K pack column offsets: [skt | gkt | qt | lkt]
KO_GKT = SKT_W               # 1344
KO_QT = KO_GKT + 128         # 1472
KO_LKT = KO_QT + T           # 5568
KW = KO_LKT + LKT_W          # 9920

# V pack column offsets: [gv | sv chunks 0-3 | lv chunks 0-9 | rest]
SV_PRE, LV_PRE = 4, 10       # prefix chunk counts (cover pairs hb 0-3)
VO_SV1 = 65
VO_LV1 = VO_SV1 + SV_PRE * 4 * 65        # 1105
VO_SVS = VO_LV1 + LV_PRE * 65            # 1755 = prefix end
VO_LVS = VO_SVS + (SV_C - SV_PRE) * 4 * 65  # 3575
VW = VO_LVS + (LV_C - LV_PRE) * 65       # 5135

F32 = mybir.dt.float32
BF16 = mybir.dt.bfloat16
GE = "sem-ge"

# column layout of the per-pair score/prob tile [128, 1536] (3 PSUM banks;
# regions never cross a 512-col bank boundary)
C_SP1A, C_SP1B = 0, 128
C_SP2A, C_SP2B = 256, 384
C_G = 512        # 256 wide: q of both blocks
C_LOC1 = 768     # 256 wide: local chunk b+1, both blocks
C_LOC0 = 1024    # 128: local chunk b, block A only
C_LOC2 = 1152    # 256 wide: local chunk b+2, both blocks
C_LOC3 = 1408    # 128: local chunk b+3, block B only


def _sv_col(c, q):
    if c < SV_PRE:
        return VO_SV1 + (c * 4 + q) * 65
    return VO_SVS + ((c - SV_PRE) * 4 + q) * 65


def _lv_col(c):
    if c < LV_PRE:
        return VO_LV1 + c * 65
    return VO_LVS + (c - LV_PRE) * 65


def _build_bass():
    nc = bass.Bass("TRN2", num_devices=NCORES, debug=False)

    kt = nc.dram_tensor("kt", [SL, 64, KW], BF16, kind="ExternalInput")
    vt = nc.dram_tensor("vt", [SL, 128, VW], BF16, kind="ExternalInput")
    o = nc.dram_tensor("o", [SL, T, D], F32, kind="ExternalOutput")

    EXP = mybir.ActivationFunctionType.Exp

    with ExitStack() as es:
        ec = es.enter_context
        # double-buffered inputs (slot parity); kt_t rows 64:128 are zeroed
        # once so every matmul contracts over 128 partitions (full PE rate)
        kt_t = [ec(nc.sbuf_tensor(f"kt_t{i}", [128, KW], BF16)) for i in range(2)]
        vt_t = [ec(nc.sbuf_tensor(f"vt_t{i}", [128, VW], BF16)) for i in range(2)]
        # double-buffered per-pair working set (pair parity)
        psS = [ec(nc.psum_tensor(f"psS{i}", [128, 1536], F32)) for i in range(2)]  # 3 banks
        pv = [ec(nc.psum_tensor(f"pv{i}", [128, 512], F32)) for i in range(2)]     # 1 bank
        pp = [ec(nc.sbuf_tensor(f"pp{i}", [128, 1536], BF16)) for i in range(2)]
        rec = [ec(nc.sbuf_tensor(f"rec{i}", [128, 2], F32)) for i in range(2)]
        # 16-deep output ring: slot-load DMA bursts delay store packets by
        # up to ~12us in the shared engines; 16 pairs of slack rides it out.
        OBN = 16
        ob = [ec(nc.sbuf_tensor(f"ob{i}", [128, 128], F32)) for i in range(OBN)]

        diK = [ec(nc.semaphore(f"diK{i}")) for i in range(2)]  # K pack, slot parity
        diV = [ec(nc.semaphore(f"diV{i}")) for i in range(2)]  # V pack, slot parity
        diP = ec(nc.semaphore("diP"))    # slot-0 V prefix (pairs 0-3)
        st = ec(nc.semaphore("st"))      # out stores (+16 per store, FIFO)
        izv = ec(nc.semaphore("izv"))    # kt_t[0] rows 64:128 zeroed (DVE)
        iza = ec(nc.semaphore("iza"))    # kt_t[1] rows 64:128 zeroed (ACT)
        pe_s = ec(nc.semaphore("pe_s"))  # +2 per pair: score matmuls (6, 9) done
        pe_v = ec(nc.semaphore("pe_v"))  # +1 per pair: PV matmuls done
        act = ec(nc.semaphore("act"))    # +1 per pair: exp done
        dve = ec(nc.semaphore("dve"))    # +1 per pair: normalize done
        block = ec(nc.Block(no_gpsimd_drain=True))

        @block.gpsimd
        def _(gpsimd):
            # slot 0: K pack, V prefix (pairs 0-3), V suffix
            gpsimd.dma_start(kt_t[0][0:64, :], kt[0]).then_inc(diK[0], 16)
            gpsimd.dma_start(vt_t[0][:, 0:VO_SVS], vt[0][:, 0:VO_SVS]).then_inc(diP, 16)
            gpsimd.dma_start(vt_t[0][:, VO_SVS:VW], vt[0][:, VO_SVS:VW]).then_inc(diV[0], 16)
            # slots 1-3 in 4 pieces, each gated one pair apart: an unthrottled
            # multi-MB burst monopolizes the DMA engines long enough to
            # starve the PE sequencer's own instruction-fetch DMAs.
            KH, VH = KW // 2, VW // 2
            for s, g in ((1, 1), (2, 16), (3, 32)):
                u = s % 2
                gpsimd.dma_start(kt_t[u][0:64, 0:KH], kt[s][:, 0:KH]).wait_op(
                    pe_v, g, GE
                ).then_inc(diK[u], 16)
                gpsimd.dma_start(kt_t[u][0:64, KH:KW], kt[s][:, KH:KW]).wait_op(
                    pe_v, g + 1, GE
                ).then_inc(diK[u], 16)
                gpsimd.dma_start(vt_t[u][:, 0:VH], vt[s][:, 0:VH]).wait_op(
                    pe_v, g + 2, GE
                ).then_inc(diV[u], 16)
                gpsimd.dma_start(vt_t[u][:, VH:VW], vt[s][:, VH:VW]).wait_op(
                    pe_v, g + 3, GE
                ).then_inc(diV[u], 16)

        def emit_scores(p, lo=0, hi=9):
            s, hb = divmod(p, PPS)
            b = 2 * hb
            su = s % 2
            if lo == 0 and hb == 0:
                # slot 0 loads K in 1 piece (16), slots 1-3 in 2 pieces (32)
                nc.tensor.wait_ge(diK[su], 16 + 32 * (s // 2) if su == 0
                                  else 32 * (s // 2 + 1))
            K = kt_t[su]
            qA = K[:, KO_QT + b * B : KO_QT + (b + 1) * B]
            qB = K[:, KO_QT + (b + 1) * B : KO_QT + (b + 2) * B]
            qAB = K[:, KO_QT + b * B : KO_QT + (b + 2) * B]
            w1a, w2a = 32 * b, 32 * b + 224
            w1b, w2b = w1a + 32, w2a + 32
            u = p % 2
            lo_l = KO_LKT
            mms = (
                (C_SP1A, 128, K[:, w1a : w1a + 128], qA),
                (C_SP1B, 128, K[:, w1b : w1b + 128], qB),
                (C_SP2A, 128, K[:, w2a : w2a + 128], qA),
                (C_SP2B, 128, K[:, w2b : w2b + 128], qB),
                (C_G, 256, K[:, KO_GKT : KO_GKT + 128], qAB),
                (C_LOC1, 256, K[:, lo_l + (b + 1) * B : lo_l + (b + 2) * B], qAB),
                (C_LOC0, 128, K[:, lo_l + b * B : lo_l + (b + 1) * B], qA),
                (C_LOC2, 256, K[:, lo_l + (b + 2) * B : lo_l + (b + 3) * B], qAB),
                (C_LOC3, 128, K[:, lo_l + (b + 3) * B : lo_l + (b + 4) * B], qB),
            )
            for kk in range(lo, hi):
                col, w, lhsT, rhs = mms[kk]
                mm = nc.tensor.matmul(
                    psS[u][:, col : col + w],
                    lhsT, rhs,
                    start=True, stop=True,
                )
                if kk == 5 or kk == 8:
                    mm.then_inc(pe_s, 1)

        def pv_mms(p):
            s, hb = divmod(p, PPS)
            b = 2 * hb
            u = p % 2
            su = s % 2
            V = vt_t[su]
            sp = []
            for blk in range(2):
                bb = b + blk
                w1, w2 = 32 * bb, 32 * bb + 224
                c1, r1 = divmod(w1, 128)
                c2, r2 = divmod(w2, 128)
                sp.append((_sv_col(c1, r1 // 32), _sv_col(c2, r2 // 32)))
            lv = [_lv_col(b + k) for k in range(4)]
            outA = pv[u][:, 0:65]
            outB = pv[u][:, 128:193]
            # Sequential accumulation groups (A fully, then B): a start=True
            # marks the surrounding 2KB PSUM zero-region pending-zero, so two
            # interleaved in-flight groups in one bank corrupt each other.
            # (out, pp col, rhs, start, stop)
            return (
                (outA, C_SP1A, V[:, sp[0][0] : sp[0][0] + 65], True, False),
                (outA, C_SP2A, V[:, sp[0][1] : sp[0][1] + 65], False, False),
                (outA, C_G, V[:, 0:65], False, False),
                (outA, C_LOC1, V[:, lv[1] : lv[1] + 65], False, False),
                (outA, C_LOC0, V[:, lv[0] : lv[0] + 65], False, False),
                (outA, C_LOC2, V[:, lv[2] : lv[2] + 65], False, True),
                (outB, C_SP1B, V[:, sp[1][0] : sp[1][0] + 65], True, False),
                (outB, C_SP2B, V[:, sp[1][1] : sp[1][1] + 65], False, False),
                (outB, C_G + 128, V[:, 0:65], False, False),
                (outB, C_LOC1 + 128, V[:, lv[1] : lv[1] + 65], False, False),
                (outB, C_LOC2 + 128, V[:, lv[2] : lv[2] + 65], False, False),
                (outB, C_LOC3, V[:, lv[3] : lv[3] + 65], False, True),
            )

        def emit_pv_range(p, mms, lo, hi):
            u = p % 2
            for kk in range(lo, hi):
                out, col, rhs, st_, sp_ = mms[kk]
                mm = nc.tensor.matmul(
                    out, pp[u][:, col : col + 128], rhs,
                    start=st_, stop=sp_, skip_group_check=True,
                )
                if kk == 11:
                    mm.then_inc(pe_v, 1)

        @block.tensor
        def _(tensor):
            tensor.wait_ge(izv, 1)
            tensor.wait_ge(iza, 1)
            tensor.wait_ge(diK[0], 16)
            emit_scores(0)
            emit_scores(1)
            for p in range(NP):
                s, hb = divmod(p, PPS)
                su = s % 2
                if p >= 2:
                    tensor.wait_ge(dve, p - 1)  # pv[u] free
                if s == 0:
                    if hb == 0:
                        tensor.wait_ge(diP, 16)      # V prefix: pairs 0-3
                    elif hb == 4:
                        tensor.wait_ge(diV[0], 16)   # slot-0 V suffix
                elif hb == 0:
                    # slot 0 V is 1 diV inc (16), slots 1-3 are 2 (32)
                    tensor.wait_ge(diV[su], 16 + 32 * (s // 2) if su == 0
                                   else 32 * (s // 2 + 1))
                mms = pv_mms(p)
                # everything below needs only exp(p) done; scores(p+2) are
                # interleaved so pe_s fires mid-iteration, keeping ACT fed
                tensor.wait_ge(act, p + 1)
                emit_pv_range(p, mms, 0, 4)   # A: sp1 sp2 G loc1
                if p + 2 < NP:
                    emit_scores(p + 2, 0, 6)  # sp x4, G, LOC1
                emit_pv_range(p, mms, 4, 12)  # A: loc0 loc2; B: all
                if p + 2 < NP:
                    emit_scores(p + 2, 6, 9)  # LOC0 LOC2 LOC3

        @block.scalar
        def _(scalar):
            nc.scalar.memzero(kt_t[1][64:128, :]).then_inc(iza, 1)
            # one exp per pair: ACT is the pacer and each ACTIVATE pays a
            # 143ns PSUM-access bubble, so a single op is cheapest.  No pe_v
            # wait needed: pe_s >= 2p+2 means scores(p) mm8 is done, which
            # the in-order PE completed after PV(p-2)'s last matmul, so
            # pp[u] is already free.
            for p in range(NP):
                u = p % 2
                nc.scalar.activation(
                    pp[u][:, 0:1536], psS[u][:, 0:1536], EXP, scale=0.125
                ).wait_op(pe_s, 2 * p + 2, GE).then_inc(act, 1)

        @block.vector
        def _(vector):
            nc.vector.memzero(kt_t[0][64:128, :]).then_inc(izv, 1)
            for p in range(NP):
                u = p % 2
                w = p % OBN
                if p >= OBN:
                    vector.wait_ge(st, 16 * (p - OBN + 1))  # ob[w] stored
                nc.vector.reciprocal(rec[u][:, 0:1], pv[u][:, 64:65]).wait_op(
                    pe_v, p + 1, GE
                )
                nc.vector.reciprocal(rec[u][:, 1:2], pv[u][:, 192:193])
                nc.vector.drain()  # DVE pipeline RAW: rec written, read next
                nc.vector.tensor_mul(
                    ob[w][:, 0:64], pv[u][:, 0:64],
                    rec[u][:, 0:1].broadcast_to([128, 64]),
                )
                nc.vector.tensor_mul(
                    ob[w][:, 64:128], pv[u][:, 128:192],
                    rec[u][:, 1:2].broadcast_to([128, 64]),
                ).then_inc(dve, 1)

        @block.sync
        def _(sync):
            for p in range(NP):
                s, hb = divmod(p, PPS)
                b = 2 * hb
                dst = o[s, b * B : (b + 2) * B, :].rearrange(
                    "(blk q) d -> q blk d", blk=2
                )
                src = ob[p % OBN][:, 0:128].rearrange("q (blk d) -> q blk d", blk=2)
                sync.dma_start(dst, src).wait_op(dve, p + 1, GE).then_inc(st, 16)
            sync.wait_ge(st, 16 * NP)

    return nc


def _prepare(inputs):
    import ml_dtypes

    bf = ml_dtypes.bfloat16
    f = np.float32
    q = np.asarray(inputs["query_layer"], f).reshape(NH, T, D)
    k = np.asarray(inputs["key_layer"], f).reshape(NH, T, D)
    v = np.asarray(inputs["value_layer"], f).reshape(NH, T, D)
    sk = np.asarray(inputs["sparse_key"], f).reshape(NH, TSP, D)
    svv = np.asarray(inputs["sparse_value"], f).reshape(NH, TSP, D)
    gk = np.asarray(inputs["global_key"], f).reshape(NH, G, D)
    gvv = np.asarray(inputs["global_value"], f).reshape(NH, G, D)
    am = np.repeat(np.asarray(inputs["attention_mask"], f)[:, 0, 0, :], H, 0)
    sm = np.repeat(np.asarray(inputs["sparse_mask"], f)[:, 0, 0, :], H, 0)
    gm = np.repeat(np.asarray(inputs["global_mask"], f)[:, 0, 0, :], H, 0)

    # K-side pack [skt | gkt | qt | lkt], all K^T / Q^T as [64, tokens]
    kpack = np.zeros((NH, 64, KW), f)
    kpack[:, :, 160 : 160 + TSP] = sk.transpose(0, 2, 1)
    kpack[:, :, KO_GKT : KO_GKT + G] = gk.transpose(0, 2, 1)
    kpack[:, :, KO_QT : KO_QT + T] = q.transpose(0, 2, 1)
    kpack[:, :, KO_LKT + B : KO_LKT + B + T] = k.transpose(0, 2, 1)
    kpack = kpack.astype(bf)

    # V_aug rows scaled by exp(mask); pad rows are all-zero
    em_l = np.zeros((NH, LKT_W), f)
    em_l[:, B : B + T] = np.exp(am)
    lvp = np.zeros((NH, LKT_W, 65), f)
    lvp[:, B : B + T, :64] = v
    lvp[:, :, 64] = 1.0
    lvp *= em_l[:, :, None]
    lvp = np.ascontiguousarray(
        lvp.reshape(NH, LV_C, 128, 65).transpose(0, 2, 1, 3)
    ).reshape(NH, 128, LV_C * 65)

    SVP_W = 96 + SV_C * 128
    em_s = np.zeros((NH, SVP_W), f)
    em_s[:, 160 : 160 + TSP] = np.exp(sm)
    sv_pad = np.zeros((NH, SVP_W, 65), f)
    sv_pad[:, 160 : 160 + TSP, :64] = svv
    sv_pad[:, :, 64] = 1.0
    sv_pad *= em_s[:, :, None]
    svp = np.empty((NH, 4, 128, SV_C, 65), f)
    for ph in range(4):
        svp[:, ph] = (
            sv_pad[:, 32 * ph : 32 * ph + SV_C * 128]
            .reshape(NH, SV_C, 128, 65)
            .transpose(0, 2, 1, 3)
        )
    # chunk-major [chunk, phase, 65] so the chunks 0-3 prefix is contiguous
    svp = np.ascontiguousarray(svp.transpose(0, 2, 3, 1, 4))  # [NH,128,SV_C,4,65]

    gvp = np.zeros((NH, 128, 65), f)
    gvp[:, :G, :64] = gvv
    gvp[:, :G, 64] = 1.0
    gvp[:, :G] *= np.exp(gm)[:, :, None]

    vpack = np.concatenate(
        [
            gvp,
            svp[:, :, :SV_PRE].reshape(NH, 128, SV_PRE * 4 * 65),
            lvp[:, :, : LV_PRE * 65],
            svp[:, :, SV_PRE:].reshape(NH, 128, (SV_C - SV_PRE) * 4 * 65),
            lvp[:, :, LV_PRE * 65 :],
        ],
        axis=2,
    ).astype(bf)

    return [
        {
            "kt": kpack[c * SL : (c + 1) * SL],
            "vt": vpack[c * SL : (c + 1) * SL],
        }
        for c in range(NCORES)
    ]


_NC_CACHE = {}
LAST_RESULTS = None


def kernel(**inputs):
    global LAST_RESULTS
    if "nc" not in _NC_CACHE:
        _NC_CACHE["nc"] = _build_bass()
    nc = _NC_CACHE["nc"]
    in_maps = _prepare(inputs)
    res = run_bass_kernel_spmd(nc, in_maps, core_ids=list(range(NCORES)))
    LAST_RESULTS = res
    out = np.empty((NH, T, D), np.float32)
    for c in range(NCORES):
        out[c * SL : (c + 1) * SL] = res.results[c]["o"]
    return out.reshape(N, H, T, D)


# revision 45
# speedup vs baseline: 1.0729x; 1.0123x over previous
"""Block-local sparse attention (LSG-style) on 8 TRN2 NeuronCores.

Sharding: the 32 (n, h) pairs are split 4-per-core (data/head parallel, no
collectives). Host-side numpy prep re-lays-out the inputs so the device
kernel needs no transposes, all bf16:

  - K-side pack kt[s] = [64, skt | gkt | qt | lkt]: sparse/global/local K^T
    (token-padded with zeros) and Q^T, one DMA per slot.
  - V-side pack vt[s] = [128, gv | sv_pre | lv_pre | sv_suf | lv_suf]:
    V with a ones column appended (col 64), chunked [128, c, 65], every row
    scaled by exp(mask): softmax(QK/8 + m) @ V is computed as
    sum_t exp(s_t) e^{m_t} [V_t, 1], then a divide by the accumulated last
    column - exact for any additive mask, and pad tokens (e^{m}=0) vanish
    from both numerator and denominator, so no mask row and no
    max-subtraction are needed (|QK|/8 is O(5), well within fp32 exp range).
    sv holds 4 phase-shifted copies (chunk-major: [chunk, phase, 65]) so the
    32-token-granular sparse windows always start at partition 0; the
    chunks-0-3 + lv-chunks-0-9 prefix is contiguous, so one ~450KB DMA
    unblocks pairs 0-3 early in slot 0.

Key discovered hardware behavior: matmuls with 64-partition operands run
the PE in a half-array row-group mode at HALF the streaming rate, and
switching modes drains the array (~200ns). All score operands are
therefore zero-padded to 128 partitions (rows 64:128 of kt_t zeroed once
at startup by DVE/ACT, overlapped with the initial loads); every matmul
then streams 1 column / 0.417ns.

The device processes query-block PAIRS: 9 score matmuls per pair into a
3-bank PSUM region [128, 1536] (no matmul output crosses a bank), one
exp(S/8) on ACT (the pacing engine: cols x 0.833ns + one 143ns
PSUM-access bubble), then 12 PV matmuls (N=65) into [q, V|Z] and a
reciprocal-normalize on DVE.  Scores of pair p+2 are interleaved into
PV(p) so pe_s fires mid-iteration and ACT is never input-starved; the
ACT's pp-buffer WAR gate is implied by pe_s (in-order PE), so the exp
carries a single semaphore wait and the ACT chain runs back-to-back.

Raw bass with hand-placed semaphores (walrus: at most one sem wait per
matmul/ACT instruction). Queue assignment: input loads on the GpSimd
queue (3 DMAs per slot), output stores (one merged DMA per pair) on the
Sync queue, so stores never queue behind multi-MB loads; an 8-deep ob
ring rides out store-packet delays behind slot-load bursts in the shared
DMA engines.
"""

from contextlib import ExitStack

import numpy as np

import concourse.bass as bass
import concourse.mybir as mybir
from concourse.bass_utils import run_bass_kernel_spmd

N, H, T, D = 2, 16, 4096, 64
B = 128          # query block
NB = T // B      # 32
G = 64           # global tokens
TSP = T // 4     # sparse tokens (1024)
NH = N * H       # 32
NCORES = 8
SL = NH // NCORES  # 4 heads per core
NP = SL * NB // 2  # 64 block-pairs per core
PPS = NB // 2      # 16 pairs per slot

LKT_W = T + 2 * B            # 4352 padded local tokens
SKT_W = TSP + 320            # 1344 padded sparse tokens
LV_C = LKT_W // 128          # 34 local V chunks
SV_C = 11                    # sparse V chunks per phase

# K pack column offsets: [skt | gkt | qt1 | lkt1 | qt2 | lkt2] where
# qt1 = q cols 0:2048 (pairs hb<8), lkt1 = local cols 0:2560 (pairs hb<9),
# qt2/lkt2 the remainders (lkt2 re-starts at col 2304: 256-col halo dup so
# no pair's 4-chunk window straddles the piece boundary).  The first-piece
# prefix [0:KP1) unblocks scores of pairs 0-7 after ~760KB instead of 1.3MB.
KO_GKT = SKT_W               # 1344
KO_QT1 = KO_GKT + 128        # 1472
KO_LKT1 = KO_QT1 + 2048      # 3520
KP1 = KO_LKT1 + 2560         # 6080 = end of piece 1
KO_QT2 = KP1                 # 6080 (q cols 2048:4096)
KO_LKT2 = KO_QT2 + 2048      # 8128 (local cols 2304:4352)
KW = KO_LKT2 + 2048          # 10176

# V pack column offsets: [gv | sv chunks 0-3 | lv chunks 0-9 | rest]
SV_PRE, LV_PRE = 4, 10       # prefix chunk counts (cover pairs hb 0-3)
VO_SV1 = 65
VO_LV1 = VO_SV1 + SV_PRE * 4 * 65        # 1105
VO_SVS = VO_LV1 + LV_PRE * 65            # 1755 = prefix end
VO_LVS = VO_SVS + (SV_C - SV_PRE) * 4 * 65  # 3575
VW = VO_LVS + (LV_C - LV_PRE) * 65       # 5135

F32 = mybir.dt.float32
BF16 = mybir.dt.bfloat16
GE = "sem-ge"

# column layout of the per-pair score/prob tile [128, 1536] (3 PSUM banks;
# regions never cross a 512-col bank boundary)
C_SP1A, C_SP1B = 0, 128
C_SP2A, C_SP2B = 256, 384
C_G = 512        # 256 wide: q of both blocks
C_LOC1 = 768     # 256 wide: local chunk b+1, both blocks
C_LOC0 = 1024    # 128: local chunk b, block A only
C_LOC2 = 1152    # 256 wide: local chunk b+2, both blocks
C_LOC3 = 1408    # 128: local chunk b+3, block B only


def _sv_col(c, q):
    if c < SV_PRE:
        return VO_SV1 + (c * 4 + q) * 65
    return VO_SVS + ((c - SV_PRE) * 4 + q) * 65


def _lv_col(c):
    if c < LV_PRE:
        return VO_LV1 + c * 65
    return VO_LVS + (c - LV_PRE) * 65


def _build_bass():
    nc = bass.Bass("TRN2", num_devices=NCORES, debug=False)

    kt = nc.dram_tensor("kt", [SL, 64, KW], BF16, kind="ExternalInput")
    vt = nc.dram_tensor("vt", [SL, 128, VW], BF16, kind="ExternalInput")
    o = nc.dram_tensor("o", [SL, T, D], F32, kind="ExternalOutput")

    EXP = mybir.ActivationFunctionType.Exp

    with ExitStack() as es:
        ec = es.enter_context
        # double-buffered inputs (slot parity); kt_t rows 64:128 are zeroed
        # once so every matmul contracts over 128 partitions (full PE rate)
        kt_t = [ec(nc.sbuf_tensor(f"kt_t{i}", [128, KW], BF16)) for i in range(2)]
        vt_t = [ec(nc.sbuf_tensor(f"vt_t{i}", [128, VW], BF16)) for i in range(2)]
        # double-buffered per-pair working set (pair parity)
        psS = [ec(nc.psum_tensor(f"psS{i}", [128, 1536], F32)) for i in range(2)]  # 3 banks
        pv = [ec(nc.psum_tensor(f"pv{i}", [128, 512], F32)) for i in range(2)]     # 1 bank
        pp = [ec(nc.sbuf_tensor(f"pp{i}", [128, 1536], BF16)) for i in range(2)]
        rec = [ec(nc.sbuf_tensor(f"rec{i}", [128, 2], F32)) for i in range(2)]
        # 16-deep output ring: slot-load DMA bursts delay store packets by
        # up to ~12us in the shared engines; 16 pairs of slack rides it out.
        OBN = 16
        ob = [ec(nc.sbuf_tensor(f"ob{i}", [128, 128], F32)) for i in range(OBN)]

        diK = [ec(nc.semaphore(f"diK{i}")) for i in range(2)]  # K pack, slot parity
        diV = [ec(nc.semaphore(f"diV{i}")) for i in range(2)]  # V pack, slot parity
        diP = ec(nc.semaphore("diP"))    # slot-0 V prefix (pairs 0-3)
        st = ec(nc.semaphore("st"))      # out stores (+16 per store, FIFO)
        izv = ec(nc.semaphore("izv"))    # kt_t[0] rows 64:128 zeroed (DVE)
        iza = ec(nc.semaphore("iza"))    # kt_t[1] rows 64:128 zeroed (ACT)
        pe_s = ec(nc.semaphore("pe_s"))  # +2 per pair: score matmuls (6, 9) done
        pe_v = ec(nc.semaphore("pe_v"))  # +1 per pair: PV matmuls done
        act = ec(nc.semaphore("act"))    # +1 per pair: exp done
        dve = ec(nc.semaphore("dve"))    # +1 per pair: normalize done
        block = ec(nc.Block(no_gpsimd_drain=True))

        @block.gpsimd
        def _(gpsimd):
            # slot 0, ordered by first use: K piece1 (scores 0-7), V prefix
            # (PV 0-3), then the remainders
            gpsimd.dma_start(kt_t[0][0:64, 0:KP1], kt[0][:, 0:KP1]).then_inc(diK[0], 16)
            gpsimd.dma_start(vt_t[0][:, 0:VO_SVS], vt[0][:, 0:VO_SVS]).then_inc(diP, 16)
            gpsimd.dma_start(kt_t[0][0:64, KP1:KO_LKT2], kt[0][:, KP1:KO_LKT2]).then_inc(diK[0], 16)
            gpsimd.dma_start(kt_t[0][0:64, KO_LKT2:KW], kt[0][:, KO_LKT2:KW]).then_inc(diK[0], 16)
            gpsimd.dma_start(vt_t[0][:, VO_SVS:VW], vt[0][:, VO_SVS:VW]).then_inc(diV[0], 16)
            # slots 1-3 in 5 pieces, each gated one pair apart: an
            # unthrottled multi-MB burst monopolizes the DMA engines long
            # enough to starve the PE sequencer's instruction-fetch DMAs.
            VH = VW // 2
            for s, g in ((1, 1), (2, 16), (3, 32)):
                u = s % 2
                for i, (t0, t1) in enumerate(
                    ((0, KP1), (KP1, KO_LKT2), (KO_LKT2, KW))
                ):
                    gpsimd.dma_start(
                        kt_t[u][0:64, t0:t1], kt[s][:, t0:t1]
                    ).wait_op(pe_v, g + i, GE).then_inc(diK[u], 16)
                gpsimd.dma_start(vt_t[u][:, 0:VH], vt[s][:, 0:VH]).wait_op(
                    pe_v, g + 3, GE
                ).then_inc(diV[u], 16)
                gpsimd.dma_start(vt_t[u][:, VH:VW], vt[s][:, VH:VW]).wait_op(
                    pe_v, g + 4, GE
                ).then_inc(diV[u], 16)

        def emit_scores(p, lo=0, hi=9):
            s, hb = divmod(p, PPS)
            b = 2 * hb
            su = s % 2
            if lo == 0:
                # every slot loads K in 3 pieces of 16: piece1 covers
                # scores of pairs 0-7, qt2 pairs 8+, lkt2 pairs 9+
                if hb == 0:
                    nc.tensor.wait_ge(diK[su], 48 * (s // 2) + 16)
                elif hb == 8:
                    nc.tensor.wait_ge(diK[su], 48 * (s // 2) + 32)
                elif hb == 9:
                    nc.tensor.wait_ge(diK[su], 48 * (s // 2) + 48)
            K = kt_t[su]
            qo = KO_QT1 if b <= 14 else KO_QT2 - 2048
            qA = K[:, qo + b * B : qo + (b + 1) * B]
            qB = K[:, qo + (b + 1) * B : qo + (b + 2) * B]
            qAB = K[:, qo + b * B : qo + (b + 2) * B]
            w1a, w2a = 32 * b, 32 * b + 224
            w1b, w2b = w1a + 32, w2a + 32
            u = p % 2
            lo_l = KO_LKT1 if b <= 16 else KO_LKT2 - 2304
            mms = (
                (C_SP1A, 128, K[:, w1a : w1a + 128], qA),
                (C_SP1B, 128, K[:, w1b : w1b + 128], qB),
                (C_SP2A, 128, K[:, w2a : w2a + 128], qA),
                (C_SP2B, 128, K[:, w2b : w2b + 128], qB),
                (C_G, 256, K[:, KO_GKT : KO_GKT + 128], qAB),
                (C_LOC1, 256, K[:, lo_l + (b + 1) * B : lo_l + (b + 2) * B], qAB),
                (C_LOC0, 128, K[:, lo_l + b * B : lo_l + (b + 1) * B], qA),
                (C_LOC2, 256, K[:, lo_l + (b + 2) * B : lo_l + (b + 3) * B], qAB),
                (C_LOC3, 128, K[:, lo_l + (b + 3) * B : lo_l + (b + 4) * B], qB),
            )
            for kk in range(lo, hi):
                col, w, lhsT, rhs = mms[kk]
                mm = nc.tensor.matmul(
                    psS[u][:, col : col + w],
                    lhsT, rhs,
                    start=True, stop=True,
                )
                if kk == 5 or kk == 8:
                    mm.then_inc(pe_s, 1)

        def pv_mms(p):
            s, hb = divmod(p, PPS)
            b = 2 * hb
            u = p % 2
            su = s % 2
            V = vt_t[su]
            sp = []
            for blk in range(2):
                bb = b + blk
                w1, w2 = 32 * bb, 32 * bb + 224
                c1, r1 = divmod(w1, 128)
                c2, r2 = divmod(w2, 128)
                sp.append((_sv_col(c1, r1 // 32), _sv_col(c2, r2 // 32)))
            lv = [_lv_col(b + k) for k in range(4)]
            outA = pv[u][:, 0:65]
            outB = pv[u][:, 128:193]
            # Sequential accumulation groups (A fully, then B): a start=True
            # marks the surrounding 2KB PSUM zero-region pending-zero, so two
            # interleaved in-flight groups in one bank corrupt each other.
            # (out, pp col, rhs, start, stop)
            return (
                (outA, C_SP1A, V[:, sp[0][0] : sp[0][0] + 65], True, False),
                (outA, C_SP2A, V[:, sp[0][1] : sp[0][1] + 65], False, False),
                (outA, C_G, V[:, 0:65], False, False),
                (outA, C_LOC1, V[:, lv[1] : lv[1] + 65], False, False),
                (outA, C_LOC0, V[:, lv[0] : lv[0] + 65], False, False),
                (outA, C_LOC2, V[:, lv[2] : lv[2] + 65], False, True),
                (outB, C_SP1B, V[:, sp[1][0] : sp[1][0] + 65], True, False),
                (outB, C_SP2B, V[:, sp[1][1] : sp[1][1] + 65], False, False),
                (outB, C_G + 128, V[:, 0:65], False, False),
                (outB, C_LOC1 + 128, V[:, lv[1] : lv[1] + 65], False, False),
                (outB, C_LOC2 + 128, V[:, lv[2] : lv[2] + 65], False, False),
                (outB, C_LOC3, V[:, lv[3] : lv[3] + 65], False, True),
            )

        def emit_pv_range(p, mms, lo, hi):
            u = p % 2
            for kk in range(lo, hi):
                out, col, rhs, st_, sp_ = mms[kk]
                mm = nc.tensor.matmul(
                    out, pp[u][:, col : col + 128], rhs,
                    start=st_, stop=sp_, skip_group_check=True,
                )
                if kk == 11:
                    mm.then_inc(pe_v, 1)

        @block.tensor
        def _(tensor):
            tensor.wait_ge(izv, 1)
            tensor.wait_ge(iza, 1)
            tensor.wait_ge(diK[0], 16)
            emit_scores(0)
            emit_scores(1)
            for p in range(NP):
                s, hb = divmod(p, PPS)
                su = s % 2
                if p >= 2:
                    tensor.wait_ge(dve, p - 1)  # pv[u] free
                if s == 0:
                    if hb == 0:
                        tensor.wait_ge(diP, 16)      # V prefix: pairs 0-3
                    elif hb == 4:
                        tensor.wait_ge(diV[0], 16)   # slot-0 V suffix
                elif hb == 0:
                    # slot 0 V is 1 diV inc (16), slots 1-3 are 2 (32)
                    tensor.wait_ge(diV[su], 16 + 32 * (s // 2) if su == 0
                                   else 32 * (s // 2 + 1))
                mms = pv_mms(p)
                # everything below needs only exp(p) done; scores(p+2) are
                # interleaved so pe_s fires mid-iteration, keeping ACT fed
                tensor.wait_ge(act, p + 1)
                emit_pv_range(p, mms, 0, 4)   # A: sp1 sp2 G loc1
                if p + 2 < NP:
                    emit_scores(p + 2, 0, 6)  # sp x4, G, LOC1
                emit_pv_range(p, mms, 4, 12)  # A: loc0 loc2; B: all
                if p + 2 < NP:
                    emit_scores(p + 2, 6, 9)  # LOC0 LOC2 LOC3

        @block.scalar
        def _(scalar):
            nc.scalar.memzero(kt_t[1][64:128, :]).then_inc(iza, 1)
            # one exp per pair: ACT is the pacer and each ACTIVATE pays a
            # 143ns PSUM-access bubble, so a single op is cheapest.  No pe_v
            # wait needed: pe_s >= 2p+2 means scores(p) mm8 is done, which
            # the in-order PE completed after PV(p-2)'s last matmul, so
            # pp[u] is already free.
            for p in range(NP):
                u = p % 2
                nc.scalar.activation(
                    pp[u][:, 0:1536], psS[u][:, 0:1536], EXP, scale=0.125
                ).wait_op(pe_s, 2 * p + 2, GE).then_inc(act, 1)

        @block.vector
        def _(vector):
            nc.vector.memzero(kt_t[0][64:128, :]).then_inc(izv, 1)
            for p in range(NP):
                u = p % 2
                w = p % OBN
                if p >= OBN:
                    vector.wait_ge(st, 16 * (p - OBN + 1))  # ob[w] stored
                nc.vector.reciprocal(rec[u][:, 0:1], pv[u][:, 64:65]).wait_op(
                    pe_v, p + 1, GE
                )
                nc.vector.reciprocal(rec[u][:, 1:2], pv[u][:, 192:193])
                nc.vector.drain()  # DVE pipeline RAW: rec written, read next
                nc.vector.tensor_mul(
                    ob[w][:, 0:64], pv[u][:, 0:64],
                    rec[u][:, 0:1].broadcast_to([128, 64]),
                )
                nc.vector.tensor_mul(
                    ob[w][:, 64:128], pv[u][:, 128:192],
                    rec[u][:, 1:2].broadcast_to([128, 64]),
                ).then_inc(dve, 1)

        @block.sync
        def _(sync):
            for p in range(NP):
                s, hb = divmod(p, PPS)
                b = 2 * hb
                dst = o[s, b * B : (b + 2) * B, :].rearrange(
                    "(blk q) d -> q blk d", blk=2
                )
                src = ob[p % OBN][:, 0:128].rearrange("q (blk d) -> q blk d", blk=2)
                sync.dma_start(dst, src).wait_op(dve, p + 1, GE).then_inc(st, 16)
            sync.wait_ge(st, 16 * NP)

    return nc


def _prepare(inputs):
    import ml_dtypes

    bf = ml_dtypes.bfloat16
    f = np.float32
    q = np.asarray(inputs["query_layer"], f).reshape(NH, T, D)
    k = np.asarray(inputs["key_layer"], f).reshape(NH, T, D)
    v = np.asarray(inputs["value_layer"], f).reshape(NH, T, D)
    sk = np.asarray(inputs["sparse_key"], f).reshape(NH, TSP, D)
    svv = np.asarray(inputs["sparse_value"], f).reshape(NH, TSP, D)
    gk = np.asarray(inputs["global_key"], f).reshape(NH, G, D)
    gvv = np.asarray(inputs["global_value"], f).reshape(NH, G, D)
    am = np.repeat(np.asarray(inputs["attention_mask"], f)[:, 0, 0, :], H, 0)
    sm = np.repeat(np.asarray(inputs["sparse_mask"], f)[:, 0, 0, :], H, 0)
    gm = np.repeat(np.asarray(inputs["global_mask"], f)[:, 0, 0, :], H, 0)

    # K-side pack [skt | gkt | qt1 | lkt1 | qt2 | lkt2] (see offsets above)
    qt = q.transpose(0, 2, 1)
    lktf = np.zeros((NH, 64, LKT_W), f)
    lktf[:, :, B : B + T] = k.transpose(0, 2, 1)
    kpack = np.zeros((NH, 64, KW), f)
    kpack[:, :, 160 : 160 + TSP] = sk.transpose(0, 2, 1)
    kpack[:, :, KO_GKT : KO_GKT + G] = gk.transpose(0, 2, 1)
    kpack[:, :, KO_QT1 : KO_QT1 + 2048] = qt[:, :, 0:2048]
    kpack[:, :, KO_LKT1 : KO_LKT1 + 2560] = lktf[:, :, 0:2560]
    kpack[:, :, KO_QT2 : KO_QT2 + 2048] = qt[:, :, 2048:4096]
    kpack[:, :, KO_LKT2 : KO_LKT2 + 2048] = lktf[:, :, 2304:4352]
    kpack = kpack.astype(bf)

    # V_aug rows scaled by exp(mask); pad rows are all-zero
    em_l = np.zeros((NH, LKT_W), f)
    em_l[:, B : B + T] = np.exp(am)
    lvp = np.zeros((NH, LKT_W, 65), f)
    lvp[:, B : B + T, :64] = v
    lvp[:, :, 64] = 1.0
    lvp *= em_l[:, :, None]
    lvp = np.ascontiguousarray(
        lvp.reshape(NH, LV_C, 128, 65).transpose(0, 2, 1, 3)
    ).reshape(NH, 128, LV_C * 65)

    SVP_W = 96 + SV_C * 128
    em_s = np.zeros((NH, SVP_W), f)
    em_s[:, 160 : 160 + TSP] = np.exp(sm)
    sv_pad = np.zeros((NH, SVP_W, 65), f)
    sv_pad[:, 160 : 160 + TSP, :64] = svv
    sv_pad[:, :, 64] = 1.0
    sv_pad *= em_s[:, :, None]
    svp = np.empty((NH, 4, 128, SV_C, 65), f)
    for ph in range(4):
        svp[:, ph] = (
            sv_pad[:, 32 * ph : 32 * ph + SV_C * 128]
            .reshape(NH, SV_C, 128, 65)
            .transpose(0, 2, 1, 3)
        )
    # chunk-major [chunk, phase, 65] so the chunks 0-3 prefix is contiguous
    svp = np.ascontiguousarray(svp.transpose(0, 2, 3, 1, 4))  # [NH,128,SV_C,4,65]

    gvp = np.zeros((NH, 128, 65), f)
    gvp[:, :G, :64] = gvv
    gvp[:, :G, 64] = 1.0
    gvp[:, :G] *= np.exp(gm)[:, :, None]

    vpack = np.concatenate(
        [
            gvp,
            svp[:, :, :SV_PRE].reshape(NH, 128, SV_PRE * 4 * 65),
            lvp[:, :, : LV_PRE * 65],
            svp[:, :, SV_PRE:].reshape(NH, 128, (SV_C - SV_PRE) * 4 * 65),
            lvp[:, :, LV_PRE * 65 :],
        ],
        axis=2,
    ).astype(bf)

    return [
        {
            "kt": kpack[c * SL : (c + 1) * SL],
            "vt": vpack[c * SL : (c + 1) * SL],
        }
        for c in range(NCORES)
    ]


_NC_CACHE = {}
LAST_RESULTS = None


def kernel(**inputs):
    global LAST_RESULTS
    if "nc" not in _NC_CACHE:
        _NC_CACHE["nc"] = _build_bass()
    nc = _NC_CACHE["nc"]
    in_maps = _prepare(inputs)
    res = run_bass_kernel_spmd(nc, in_maps, core_ids=list(range(NCORES)))
    LAST_RESULTS = res
    out = np.empty((NH, T, D), np.float32)
    for c in range(NCORES):
        out[c * SL : (c + 1) * SL] = res.results[c]["o"]
    return out.reshape(N, H, T, D)


# revision 47
# speedup vs baseline: 1.0853x; 1.0115x over previous
"""Block-local sparse attention (LSG-style) on 8 TRN2 NeuronCores.

Sharding: the 32 (n, h) pairs are split 4-per-core (data/head parallel, no
collectives). Host-side numpy prep re-lays-out the inputs so the device
kernel needs no transposes, all bf16:

  - K-side pack kt[s] = [64, skt | gkt | qt | lkt]: sparse/global/local K^T
    (token-padded with zeros) and Q^T, one DMA per slot.
  - V-side pack vt[s] = [128, gv | sv_pre | lv_pre | sv_suf | lv_suf]:
    V with a ones column appended (col 64), chunked [128, c, 65], every row
    scaled by exp(mask): softmax(QK/8 + m) @ V is computed as
    sum_t exp(s_t) e^{m_t} [V_t, 1], then a divide by the accumulated last
    column - exact for any additive mask, and pad tokens (e^{m}=0) vanish
    from both numerator and denominator, so no mask row and no
    max-subtraction are needed (|QK|/8 is O(5), well within fp32 exp range).
    sv holds 4 phase-shifted copies (chunk-major: [chunk, phase, 65]) so the
    32-token-granular sparse windows always start at partition 0; the
    chunks-0-3 + lv-chunks-0-9 prefix is contiguous, so one ~450KB DMA
    unblocks pairs 0-3 early in slot 0.

Key discovered hardware behavior: matmuls with 64-partition operands run
the PE in a half-array row-group mode at HALF the streaming rate, and
switching modes drains the array (~200ns). All score operands are
therefore zero-padded to 128 partitions (rows 64:128 of kt_t zeroed once
at startup by DVE/ACT, overlapped with the initial loads); every matmul
then streams 1 column / 0.417ns.

The device processes query-block PAIRS: 9 score matmuls per pair into a
3-bank PSUM region [128, 1536] (no matmul output crosses a bank), one
exp(S/8) on ACT (the pacing engine: cols x 0.833ns + one 143ns
PSUM-access bubble), then 12 PV matmuls (N=65) into [q, V|Z] and a
reciprocal-normalize on DVE.  Scores of pair p+2 are interleaved into
PV(p) so pe_s fires mid-iteration and ACT is never input-starved; the
ACT's pp-buffer WAR gate is implied by pe_s (in-order PE), so the exp
carries a single semaphore wait and the ACT chain runs back-to-back.

Raw bass with hand-placed semaphores (walrus: at most one sem wait per
matmul/ACT instruction). Queue assignment: input loads on the GpSimd
queue (3 DMAs per slot), output stores (one merged DMA per pair) on the
Sync queue, so stores never queue behind multi-MB loads; an 8-deep ob
ring rides out store-packet delays behind slot-load bursts in the shared
DMA engines.
"""

from contextlib import ExitStack

import numpy as np

import concourse.bass as bass
import concourse.mybir as mybir
from concourse.bass_utils import run_bass_kernel_spmd

N, H, T, D = 2, 16, 4096, 64
B = 128          # query block
NB = T // B      # 32
G = 64           # global tokens
TSP = T // 4     # sparse tokens (1024)
NH = N * H       # 32
NCORES = 8
SL = NH // NCORES  # 4 heads per core
NP = SL * NB // 2  # 64 block-pairs per core
PPS = NB // 2      # 16 pairs per slot

LKT_W = T + 2 * B            # 4352 padded local tokens
SKT_W = TSP + 320            # 1344 padded sparse tokens
LV_C = LKT_W // 128          # 34 local V chunks
SV_C = 11                    # sparse V chunks per phase

# K pack column offsets: [skt | gkt | qt1 | lkt1 | qt2 | lkt2] where
# qt1 = q cols 0:2048 (pairs hb<8), lkt1 = local cols 0:2560 (pairs hb<9),
# qt2/lkt2 the remainders (lkt2 re-starts at col 2304: 256-col halo dup so
# no pair's 4-chunk window straddles the piece boundary).  The first-piece
# prefix [0:KP1) unblocks scores of pairs 0-7 after ~760KB instead of 1.3MB.
KO_GKT = SKT_W               # 1344
KO_QT1 = KO_GKT + 128        # 1472
KO_LKT1 = KO_QT1 + 2048      # 3520
KP1 = KO_LKT1 + 2560         # 6080 = end of piece 1
KO_QT2 = KP1                 # 6080 (q cols 2048:4096)
KO_LKT2 = KO_QT2 + 2048      # 8128 (local cols 2304:4352)
KW = KO_LKT2 + 2048          # 10176

# V pack column offsets: [gv | sv chunks 0-3 | lv chunks 0-9 | rest]
SV_PRE, LV_PRE = 4, 10       # prefix chunk counts (cover pairs hb 0-3)
VO_SV1 = 65
VO_LV1 = VO_SV1 + SV_PRE * 4 * 65        # 1105
VO_SVS = VO_LV1 + LV_PRE * 65            # 1755 = prefix end
VO_LVS = VO_SVS + (SV_C - SV_PRE) * 4 * 65  # 3575
VW = VO_LVS + (LV_C - LV_PRE) * 65       # 5135

F32 = mybir.dt.float32
BF16 = mybir.dt.bfloat16
GE = "sem-ge"

# column layout of the per-pair score/prob tile [128, 1536] (3 PSUM banks;
# regions never cross a 512-col bank boundary)
C_SP1A, C_SP1B = 0, 128
C_SP2A, C_SP2B = 256, 384
C_G = 512        # 256 wide: q of both blocks
C_LOC1 = 768     # 256 wide: local chunk b+1, both blocks
C_LOC0 = 1024    # 128: local chunk b, block A only
C_LOC2 = 1152    # 256 wide: local chunk b+2, both blocks
C_LOC3 = 1408    # 128: local chunk b+3, block B only


def _sv_col(c, q):
    if c < SV_PRE:
        return VO_SV1 + (c * 4 + q) * 65
    return VO_SVS + ((c - SV_PRE) * 4 + q) * 65


def _lv_col(c):
    if c < LV_PRE:
        return VO_LV1 + c * 65
    return VO_LVS + (c - LV_PRE) * 65


def _build_bass():
    nc = bass.Bass("TRN2", num_devices=NCORES, debug=False)

    kt = nc.dram_tensor("kt", [SL, 64, KW], BF16, kind="ExternalInput")
    vt = nc.dram_tensor("vt", [SL, 128, VW], BF16, kind="ExternalInput")
    o = nc.dram_tensor("o", [SL, T, D], F32, kind="ExternalOutput")

    EXP = mybir.ActivationFunctionType.Exp

    with ExitStack() as es:
        ec = es.enter_context
        # double-buffered inputs (slot parity); kt_t rows 64:128 are zeroed
        # once so every matmul contracts over 128 partitions (full PE rate)
        kt_t = [ec(nc.sbuf_tensor(f"kt_t{i}", [128, KW], BF16)) for i in range(2)]
        vt_t = [ec(nc.sbuf_tensor(f"vt_t{i}", [128, VW], BF16)) for i in range(2)]
        # double-buffered per-pair working set (pair parity)
        psS = [ec(nc.psum_tensor(f"psS{i}", [128, 1536], F32)) for i in range(2)]  # 3 banks
        pv = [ec(nc.psum_tensor(f"pv{i}", [128, 512], F32)) for i in range(2)]     # 1 bank
        pp = [ec(nc.sbuf_tensor(f"pp{i}", [128, 1536], BF16)) for i in range(2)]
        rec = [ec(nc.sbuf_tensor(f"rec{i}", [128, 2], F32)) for i in range(2)]
        # 16-deep output ring: slot-load DMA bursts delay store packets by
        # up to ~12us in the shared engines; 16 pairs of slack rides it out.
        OBN = 16
        ob = [ec(nc.sbuf_tensor(f"ob{i}", [128, 128], F32)) for i in range(OBN)]

        diK = [ec(nc.semaphore(f"diK{i}")) for i in range(2)]  # K pack, slot parity
        diV = [ec(nc.semaphore(f"diV{i}")) for i in range(2)]  # V pack, slot parity
        diP = ec(nc.semaphore("diP"))    # slot-0 V prefix (pairs 0-3)
        st = ec(nc.semaphore("st"))      # out stores (+16 per store, FIFO)
        izv = ec(nc.semaphore("izv"))    # kt_t[0] rows 64:128 zeroed (DVE)
        iza = ec(nc.semaphore("iza"))    # kt_t[1] rows 64:128 zeroed (ACT)
        pe_s = ec(nc.semaphore("pe_s"))  # +2 per pair: score matmuls (6, 9) done
        pe_v = ec(nc.semaphore("pe_v"))  # +1 per pair: PV matmuls done
        act = ec(nc.semaphore("act"))    # +1 per pair: exp done
        dve = ec(nc.semaphore("dve"))    # +1 per pair: normalize done
        block = ec(nc.Block(no_gpsimd_drain=True))

        @block.gpsimd
        def _(gpsimd):
            # slot 0, ordered by first use: K piece1 (scores 0-7), V prefix
            # (PV 0-3), then the remainders
            gpsimd.dma_start(kt_t[0][0:64, 0:KP1], kt[0][:, 0:KP1]).then_inc(diK[0], 16)
            gpsimd.dma_start(vt_t[0][:, 0:VO_SVS], vt[0][:, 0:VO_SVS]).then_inc(diP, 16)
            gpsimd.dma_start(kt_t[0][0:64, KP1:KO_LKT2], kt[0][:, KP1:KO_LKT2]).then_inc(diK[0], 16)
            gpsimd.dma_start(kt_t[0][0:64, KO_LKT2:KW], kt[0][:, KO_LKT2:KW]).then_inc(diK[0], 16)
            gpsimd.dma_start(vt_t[0][:, VO_SVS:VW], vt[0][:, VO_SVS:VW]).then_inc(diV[0], 16)
            # slots 1-3 in 6 pieces gated TWO pairs apart: pieces must not
            # outpace the engines' drain rate or the backlog starves the PE
            # sequencer's instruction-fetch DMAs and the store packets.
            V3 = VW // 3
            for s, g in ((1, 1), (2, 16), (3, 32)):
                u = s % 2
                for i, (t0, t1) in enumerate(
                    ((0, KP1), (KP1, KO_LKT2), (KO_LKT2, KW))
                ):
                    gpsimd.dma_start(
                        kt_t[u][0:64, t0:t1], kt[s][:, t0:t1]
                    ).wait_op(pe_v, g + 2 * i, GE).then_inc(diK[u], 16)
                for i, (t0, t1) in enumerate(
                    ((0, V3), (V3, 2 * V3), (2 * V3, VW))
                ):
                    gpsimd.dma_start(
                        vt_t[u][:, t0:t1], vt[s][:, t0:t1]
                    ).wait_op(pe_v, g + 6 + 2 * i, GE).then_inc(diV[u], 16)

        def emit_scores(p, lo=0, hi=9):
            s, hb = divmod(p, PPS)
            b = 2 * hb
            su = s % 2
            if lo == 0:
                # every slot loads K in 3 pieces of 16: piece1 covers
                # scores of pairs 0-7, qt2 pairs 8+, lkt2 pairs 9+
                if hb == 0:
                    nc.tensor.wait_ge(diK[su], 48 * (s // 2) + 16)
                elif hb == 8:
                    nc.tensor.wait_ge(diK[su], 48 * (s // 2) + 32)
                elif hb == 9:
                    nc.tensor.wait_ge(diK[su], 48 * (s // 2) + 48)
            K = kt_t[su]
            qo = KO_QT1 if b <= 14 else KO_QT2 - 2048
            qA = K[:, qo + b * B : qo + (b + 1) * B]
            qB = K[:, qo + (b + 1) * B : qo + (b + 2) * B]
            qAB = K[:, qo + b * B : qo + (b + 2) * B]
            w1a, w2a = 32 * b, 32 * b + 224
            w1b, w2b = w1a + 32, w2a + 32
            u = p % 2
            lo_l = KO_LKT1 if b <= 16 else KO_LKT2 - 2304
            mms = (
                (C_SP1A, 128, K[:, w1a : w1a + 128], qA),
                (C_SP1B, 128, K[:, w1b : w1b + 128], qB),
                (C_SP2A, 128, K[:, w2a : w2a + 128], qA),
                (C_SP2B, 128, K[:, w2b : w2b + 128], qB),
                (C_G, 256, K[:, KO_GKT : KO_GKT + 128], qAB),
                (C_LOC1, 256, K[:, lo_l + (b + 1) * B : lo_l + (b + 2) * B], qAB),
                (C_LOC0, 128, K[:, lo_l + b * B : lo_l + (b + 1) * B], qA),
                (C_LOC2, 256, K[:, lo_l + (b + 2) * B : lo_l + (b + 3) * B], qAB),
                (C_LOC3, 128, K[:, lo_l + (b + 3) * B : lo_l + (b + 4) * B], qB),
            )
            for kk in range(lo, hi):
                col, w, lhsT, rhs = mms[kk]
                mm = nc.tensor.matmul(
                    psS[u][:, col : col + w],
                    lhsT, rhs,
                    start=True, stop=True,
                )
                if kk == 5 or kk == 8:
                    mm.then_inc(pe_s, 1)

        def pv_mms(p):
            s, hb = divmod(p, PPS)
            b = 2 * hb
            u = p % 2
            su = s % 2
            V = vt_t[su]
            sp = []
            for blk in range(2):
                bb = b + blk
                w1, w2 = 32 * bb, 32 * bb + 224
                c1, r1 = divmod(w1, 128)
                c2, r2 = divmod(w2, 128)
                sp.append((_sv_col(c1, r1 // 32), _sv_col(c2, r2 // 32)))
            lv = [_lv_col(b + k) for k in range(4)]
            outA = pv[u][:, 0:65]
            outB = pv[u][:, 128:193]
            # Sequential accumulation groups (A fully, then B): a start=True
            # marks the surrounding 2KB PSUM zero-region pending-zero, so two
            # interleaved in-flight groups in one bank corrupt each other.
            # (out, pp col, rhs, start, stop)
            return (
                (outA, C_SP1A, V[:, sp[0][0] : sp[0][0] + 65], True, False),
                (outA, C_SP2A, V[:, sp[0][1] : sp[0][1] + 65], False, False),
                (outA, C_G, V[:, 0:65], False, False),
                (outA, C_LOC1, V[:, lv[1] : lv[1] + 65], False, False),
                (outA, C_LOC0, V[:, lv[0] : lv[0] + 65], False, False),
                (outA, C_LOC2, V[:, lv[2] : lv[2] + 65], False, True),
                (outB, C_SP1B, V[:, sp[1][0] : sp[1][0] + 65], True, False),
                (outB, C_SP2B, V[:, sp[1][1] : sp[1][1] + 65], False, False),
                (outB, C_G + 128, V[:, 0:65], False, False),
                (outB, C_LOC1 + 128, V[:, lv[1] : lv[1] + 65], False, False),
                (outB, C_LOC2 + 128, V[:, lv[2] : lv[2] + 65], False, False),
                (outB, C_LOC3, V[:, lv[3] : lv[3] + 65], False, True),
            )

        def emit_pv_range(p, mms, lo, hi):
            u = p % 2
            for kk in range(lo, hi):
                out, col, rhs, st_, sp_ = mms[kk]
                mm = nc.tensor.matmul(
                    out, pp[u][:, col : col + 128], rhs,
                    start=st_, stop=sp_, skip_group_check=True,
                )
                if kk == 11:
                    mm.then_inc(pe_v, 1)

        @block.tensor
        def _(tensor):
            tensor.wait_ge(izv, 1)
            tensor.wait_ge(iza, 1)
            tensor.wait_ge(diK[0], 16)
            emit_scores(0)
            emit_scores(1)
            for p in range(NP):
                s, hb = divmod(p, PPS)
                su = s % 2
                if p >= 2:
                    tensor.wait_ge(dve, p - 1)  # pv[u] free
                if s == 0:
                    if hb == 0:
                        tensor.wait_ge(diP, 16)      # V prefix: pairs 0-3
                    elif hb == 4:
                        tensor.wait_ge(diV[0], 16)   # slot-0 V suffix
                elif hb == 0:
                    # slot 0 V is 1 diV inc (16), slots 1-3 are 3 (48)
                    tensor.wait_ge(diV[su], 16 + 48 * (s // 2) if su == 0
                                   else 48 * (s // 2 + 1))
                mms = pv_mms(p)
                # everything below needs only exp(p) done; scores(p+2) are
                # interleaved so pe_s fires mid-iteration, keeping ACT fed
                tensor.wait_ge(act, p + 1)
                emit_pv_range(p, mms, 0, 4)   # A: sp1 sp2 G loc1
                if p + 2 < NP:
                    emit_scores(p + 2, 0, 6)  # sp x4, G, LOC1
                emit_pv_range(p, mms, 4, 12)  # A: loc0 loc2; B: all
                if p + 2 < NP:
                    emit_scores(p + 2, 6, 9)  # LOC0 LOC2 LOC3

        @block.scalar
        def _(scalar):
            nc.scalar.memzero(kt_t[1][64:128, :]).then_inc(iza, 1)
            # one exp per pair: ACT is the pacer and each ACTIVATE pays a
            # 143ns PSUM-access bubble, so a single op is cheapest.  No pe_v
            # wait needed: pe_s >= 2p+2 means scores(p) mm8 is done, which
            # the in-order PE completed after PV(p-2)'s last matmul, so
            # pp[u] is already free.
            for p in range(NP):
                u = p % 2
                nc.scalar.activation(
                    pp[u][:, 0:1536], psS[u][:, 0:1536], EXP, scale=0.125
                ).wait_op(pe_s, 2 * p + 2, GE).then_inc(act, 1)

        @block.vector
        def _(vector):
            nc.vector.memzero(kt_t[0][64:128, :]).then_inc(izv, 1)
            for p in range(NP):
                u = p % 2
                w = p % OBN
                if p >= OBN:
                    vector.wait_ge(st, 16 * (p - OBN + 1))  # ob[w] stored
                nc.vector.reciprocal(rec[u][:, 0:1], pv[u][:, 64:65]).wait_op(
                    pe_v, p + 1, GE
                )
                nc.vector.reciprocal(rec[u][:, 1:2], pv[u][:, 192:193])
                nc.vector.drain()  # DVE pipeline RAW: rec written, read next
                nc.vector.tensor_mul(
                    ob[w][:, 0:64], pv[u][:, 0:64],
                    rec[u][:, 0:1].broadcast_to([128, 64]),
                )
                nc.vector.tensor_mul(
                    ob[w][:, 64:128], pv[u][:, 128:192],
                    rec[u][:, 1:2].broadcast_to([128, 64]),
                ).then_inc(dve, 1)

        @block.sync
        def _(sync):
            for p in range(NP):
                s, hb = divmod(p, PPS)
                b = 2 * hb
                dst = o[s, b * B : (b + 2) * B, :].rearrange(
                    "(blk q) d -> q blk d", blk=2
                )
                src = ob[p % OBN][:, 0:128].rearrange("q (blk d) -> q blk d", blk=2)
                sync.dma_start(dst, src).wait_op(dve, p + 1, GE).then_inc(st, 16)
            sync.wait_ge(st, 16 * NP)

    return nc


def _prepare(inputs):
    import ml_dtypes

    bf = ml_dtypes.bfloat16
    f = np.float32
    q = np.asarray(inputs["query_layer"], f).reshape(NH, T, D)
    k = np.asarray(inputs["key_layer"], f).reshape(NH, T, D)
    v = np.asarray(inputs["value_layer"], f).reshape(NH, T, D)
    sk = np.asarray(inputs["sparse_key"], f).reshape(NH, TSP, D)
    svv = np.asarray(inputs["sparse_value"], f).reshape(NH, TSP, D)
    gk = np.asarray(inputs["global_key"], f).reshape(NH, G, D)
    gvv = np.asarray(inputs["global_value"], f).reshape(NH, G, D)
    am = np.repeat(np.asarray(inputs["attention_mask"], f)[:, 0, 0, :], H, 0)
    sm = np.repeat(np.asarray(inputs["sparse_mask"], f)[:, 0, 0, :], H, 0)
    gm = np.repeat(np.asarray(inputs["global_mask"], f)[:, 0, 0, :], H, 0)

    # K-side pack [skt | gkt | qt1 | lkt1 | qt2 | lkt2] (see offsets above)
    qt = q.transpose(0, 2, 1)
    lktf = np.zeros((NH, 64, LKT_W), f)
    lktf[:, :, B : B + T] = k.transpose(0, 2, 1)
    kpack = np.zeros((NH, 64, KW), f)
    kpack[:, :, 160 : 160 + TSP] = sk.transpose(0, 2, 1)
    kpack[:, :, KO_GKT : KO_GKT + G] = gk.transpose(0, 2, 1)
    kpack[:, :, KO_QT1 : KO_QT1 + 2048] = qt[:, :, 0:2048]
    kpack[:, :, KO_LKT1 : KO_LKT1 + 2560] = lktf[:, :, 0:2560]
    kpack[:, :, KO_QT2 : KO_QT2 + 2048] = qt[:, :, 2048:4096]
    kpack[:, :, KO_LKT2 : KO_LKT2 + 2048] = lktf[:, :, 2304:4352]
    kpack = kpack.astype(bf)

    # V_aug rows scaled by exp(mask); pad rows are all-zero
    em_l = np.zeros((NH, LKT_W), f)
    em_l[:, B : B + T] = np.exp(am)
    lvp = np.zeros((NH, LKT_W, 65), f)
    lvp[:, B : B + T, :64] = v
    lvp[:, :, 64] = 1.0
    lvp *= em_l[:, :, None]
    lvp = np.ascontiguousarray(
        lvp.reshape(NH, LV_C, 128, 65).transpose(0, 2, 1, 3)
    ).reshape(NH, 128, LV_C * 65)

    SVP_W = 96 + SV_C * 128
    em_s = np.zeros((NH, SVP_W), f)
    em_s[:, 160 : 160 + TSP] = np.exp(sm)
    sv_pad = np.zeros((NH, SVP_W, 65), f)
    sv_pad[:, 160 : 160 + TSP, :64] = svv
    sv_pad[:, :, 64] = 1.0
    sv_pad *= em_s[:, :, None]
    svp = np.empty((NH, 4, 128, SV_C, 65), f)
    for ph in range(4):
        svp[:, ph] = (
            sv_pad[:, 32 * ph : 32 * ph + SV_C * 128]
            .reshape(NH, SV_C, 128, 65)
            .transpose(0, 2, 1, 3)
        )
    # chunk-major [chunk, phase, 65] so the chunks 0-3 prefix is contiguous
    svp = np.ascontiguousarray(svp.transpose(0, 2, 3, 1, 4))  # [NH,128,SV_C,4,65]

    gvp = np.zeros((NH, 128, 65), f)
    gvp[:, :G, :64] = gvv
    gvp[:, :G, 64] = 1.0
    gvp[:, :G] *= np.exp(gm)[:, :, None]

    vpack = np.concatenate(
        [
            gvp,
            svp[:, :, :SV_PRE].reshape(NH, 128, SV_PRE * 4 * 65),
            lvp[:, :, : LV_PRE * 65],
            svp[:, :, SV_PRE:].reshape(NH, 128, (SV_C - SV_PRE) * 4 * 65),
            lvp[:, :, LV_PRE * 65 :],
        ],
        axis=2,
    ).astype(bf)

    return [
        {
            "kt": kpack[c * SL : (c + 1) * SL],
            "vt": vpack[c * SL : (c + 1) * SL],
        }
        for c in range(NCORES)
    ]


_NC_CACHE = {}
LAST_RESULTS = None


def kernel(**inputs):
    global LAST_RESULTS
    if "nc" not in _NC_CACHE:
        _NC_CACHE["nc"] = _build_bass()
    nc = _NC_CACHE["nc"]
    in_maps = _prepare(inputs)
    res = run_bass_kernel_spmd(nc, in_maps, core_ids=list(range(NCORES)))
    LAST_RESULTS = res
    out = np.empty((NH, T, D), np.float32)
    for c in range(NCORES):
        out[c * SL : (c + 1) * SL] = res.results[c]["o"]
    return out.reshape(N, H, T, D)


# revision 50
# speedup vs baseline: 1.1048x; 1.0180x over previous
"""Block-local sparse attention (LSG-style) on 8 TRN2 NeuronCores.

Sharding: the 32 (n, h) pairs are split 4-per-core (data/head parallel, no
collectives). Host-side numpy prep re-lays-out the inputs so the device
kernel needs no transposes, all bf16:

  - K-side pack kt[s] = [64, skt | gkt | qt | lkt]: sparse/global/local K^T
    (token-padded with zeros) and Q^T, one DMA per slot.
  - V-side pack vt[s] = [128, gv | sv_pre | lv_pre | sv_suf | lv_suf]:
    V with a ones column appended (col 64), chunked [128, c, 65], every row
    scaled by exp(mask): softmax(QK/8 + m) @ V is computed as
    sum_t exp(s_t) e^{m_t} [V_t, 1], then a divide by the accumulated last
    column - exact for any additive mask, and pad tokens (e^{m}=0) vanish
    from both numerator and denominator, so no mask row and no
    max-subtraction are needed (|QK|/8 is O(5), well within fp32 exp range).
    sv holds 4 phase-shifted copies (chunk-major: [chunk, phase, 65]) so the
    32-token-granular sparse windows always start at partition 0; the
    chunks-0-3 + lv-chunks-0-9 prefix is contiguous, so one ~450KB DMA
    unblocks pairs 0-3 early in slot 0.

Key discovered hardware behavior: matmuls with 64-partition operands run
the PE in a half-array row-group mode at HALF the streaming rate, and
switching modes drains the array (~200ns). All score operands are
therefore zero-padded to 128 partitions (rows 64:128 of kt_t zeroed once
at startup by DVE/ACT, overlapped with the initial loads); every matmul
then streams 1 column / 0.417ns.

The device processes query-block PAIRS: 9 score matmuls per pair into a
3-bank PSUM region [128, 1536] (no matmul output crosses a bank), one
exp(S/8) on ACT (the pacing engine: cols x 0.833ns + one 143ns
PSUM-access bubble), then 12 PV matmuls (N=65) into [q, V|Z] and a
reciprocal-normalize on DVE.  Scores of pair p+2 are interleaved into
PV(p) so pe_s fires mid-iteration and ACT is never input-starved; the
ACT's pp-buffer WAR gate is implied by pe_s (in-order PE), so the exp
carries a single semaphore wait and the ACT chain runs back-to-back.

Raw bass with hand-placed semaphores (walrus: at most one sem wait per
matmul/ACT instruction). Queue assignment: input loads on the GpSimd
queue (3 DMAs per slot), output stores (one merged DMA per pair) on the
Sync queue, so stores never queue behind multi-MB loads; an 8-deep ob
ring rides out store-packet delays behind slot-load bursts in the shared
DMA engines.
"""

from contextlib import ExitStack

import numpy as np

import concourse.bass as bass
import concourse.mybir as mybir
from concourse.bass_utils import run_bass_kernel_spmd

N, H, T, D = 2, 16, 4096, 64
B = 128          # query block
NB = T // B      # 32
G = 64           # global tokens
TSP = T // 4     # sparse tokens (1024)
NH = N * H       # 32
NCORES = 8
SL = NH // NCORES  # 4 heads per core
NP = SL * NB // 2  # 64 block-pairs per core
PPS = NB // 2      # 16 pairs per slot

LKT_W = T + 2 * B            # 4352 padded local tokens
SKT_W = TSP + 320            # 1344 padded sparse tokens
LV_C = LKT_W // 128          # 34 local V chunks
SV_C = 11                    # sparse V chunks per phase

# K pack column offsets: [skt | gkt | qt1 | lkt1 | qt2 | lkt2] where
# qt1 = q cols 0:2048 (pairs hb<8), lkt1 = local cols 0:2560 (pairs hb<9),
# qt2/lkt2 the remainders (lkt2 re-starts at col 2304: 256-col halo dup so
# no pair's 4-chunk window straddles the piece boundary).  The first-piece
# prefix [0:KP1) unblocks scores of pairs 0-7 after ~760KB instead of 1.3MB.
KO_GKT = SKT_W               # 1344
KO_QT1 = KO_GKT + 128        # 1472
KO_LKT1 = KO_QT1 + 2048      # 3520
KP1 = KO_LKT1 + 2560         # 6080 = end of piece 1
KO_QT2 = KP1                 # 6080 (q cols 2048:4096)
KO_LKT2 = KO_QT2 + 2048      # 8128 (local cols 2304:4352)
KW = KO_LKT2 + 2048          # 10176

# V pack column offsets: [gv | sv chunks 0-3 | lv chunks 0-9 | rest]
SV_PRE, LV_PRE = 4, 10       # prefix chunk counts (cover pairs hb 0-3)
VO_SV1 = 65
VO_LV1 = VO_SV1 + SV_PRE * 4 * 65        # 1105
VO_SVS = VO_LV1 + LV_PRE * 65            # 1755 = prefix end
VO_LVS = VO_SVS + (SV_C - SV_PRE) * 4 * 65  # 3575
VW = VO_LVS + (LV_C - LV_PRE) * 65       # 5135

F32 = mybir.dt.float32
BF16 = mybir.dt.bfloat16
GE = "sem-ge"

# column layout of the per-pair score/prob tile [128, 1536] (3 PSUM banks;
# regions never cross a 512-col bank boundary)
C_SP1A, C_SP1B = 0, 128
C_SP2A, C_SP2B = 256, 384
C_G = 512        # 256 wide: q of both blocks
C_LOC1 = 768     # 256 wide: local chunk b+1, both blocks
C_LOC0 = 1024    # 128: local chunk b, block A only
C_LOC2 = 1152    # 256 wide: local chunk b+2, both blocks
C_LOC3 = 1408    # 128: local chunk b+3, block B only


def _sv_col(c, q):
    if c < SV_PRE:
        return VO_SV1 + (c * 4 + q) * 65
    return VO_SVS + ((c - SV_PRE) * 4 + q) * 65


def _lv_col(c):
    if c < LV_PRE:
        return VO_LV1 + c * 65
    return VO_LVS + (c - LV_PRE) * 65


def _build_bass():
    nc = bass.Bass("TRN2", num_devices=NCORES, debug=False)

    kt = nc.dram_tensor("kt", [SL, 64, KW], BF16, kind="ExternalInput")
    vt = nc.dram_tensor("vt", [SL, 128, VW], BF16, kind="ExternalInput")
    o = nc.dram_tensor("o", [SL, T, D], F32, kind="ExternalOutput")

    EXP = mybir.ActivationFunctionType.Exp

    with ExitStack() as es:
        ec = es.enter_context
        # double-buffered inputs (slot parity); kt_t rows 64:128 are zeroed
        # once so every matmul contracts over 128 partitions (full PE rate)
        kt_t = [ec(nc.sbuf_tensor(f"kt_t{i}", [128, KW], BF16)) for i in range(2)]
        vt_t = [ec(nc.sbuf_tensor(f"vt_t{i}", [128, VW], BF16)) for i in range(2)]
        # double-buffered per-pair working set (pair parity)
        psS = [ec(nc.psum_tensor(f"psS{i}", [128, 1536], F32)) for i in range(2)]  # 3 banks
        pv = [ec(nc.psum_tensor(f"pv{i}", [128, 512], F32)) for i in range(2)]     # 1 bank
        pp = [ec(nc.sbuf_tensor(f"pp{i}", [128, 1536], BF16)) for i in range(2)]
        rec = [ec(nc.sbuf_tensor(f"rec{i}", [128, 2], F32)) for i in range(2)]
        # 16-deep output ring: slot-load DMA bursts delay store packets by
        # up to ~12us in the shared engines; 16 pairs of slack rides it out.
        OBN = 16
        ob = [ec(nc.sbuf_tensor(f"ob{i}", [128, 128], F32)) for i in range(OBN)]

        diK = [ec(nc.semaphore(f"diK{i}")) for i in range(2)]  # K pack, slot parity
        diV = [ec(nc.semaphore(f"diV{i}")) for i in range(2)]  # V pack, slot parity
        diP = ec(nc.semaphore("diP"))    # slot-0 V prefix (pairs 0-3)
        st = ec(nc.semaphore("st"))      # out stores (+16 per store, FIFO)
        izv = ec(nc.semaphore("izv"))    # kt_t[0] rows 64:128 zeroed (DVE)
        iza = ec(nc.semaphore("iza"))    # kt_t[1] rows 64:128 zeroed (ACT)
        pe_s = ec(nc.semaphore("pe_s"))  # +2 per pair: score matmuls (6, 9) done
        pe_v = ec(nc.semaphore("pe_v"))  # +1 per pair: PV matmuls done
        act = ec(nc.semaphore("act"))    # +1 per pair: exp done
        dve = ec(nc.semaphore("dve"))    # +1 per pair: normalize done
        block = ec(nc.Block(no_gpsimd_drain=True))

        @block.gpsimd
        def _(gpsimd):
            # slot 0, ordered by first use: K piece1 (scores 0-7), V prefix
            # (PV 0-3), then the remainders
            gpsimd.dma_start(kt_t[0][0:64, 0:KP1], kt[0][:, 0:KP1]).then_inc(diK[0], 16)
            gpsimd.dma_start(vt_t[0][:, 0:VO_SVS], vt[0][:, 0:VO_SVS]).then_inc(diP, 16)
            gpsimd.dma_start(kt_t[0][0:64, KP1:KO_LKT2], kt[0][:, KP1:KO_LKT2]).then_inc(diK[0], 16)
            gpsimd.dma_start(kt_t[0][0:64, KO_LKT2:KW], kt[0][:, KO_LKT2:KW]).then_inc(diK[0], 16)
            gpsimd.dma_start(vt_t[0][:, VO_SVS:VW], vt[0][:, VO_SVS:VW]).then_inc(diV[0], 16)
            # slots 1-3 in 12 small pieces gated one pair apart: each piece
            # is ~1us of engine time per ~1.5us pair period, so the load
            # stream never builds the backlog that starves the PE
            # sequencer's instruction-fetch DMAs and the store packets.
            kcut = [KW * i // 5 for i in range(6)]
            vcut = [VW * i // 7 for i in range(8)]
            for s, g in ((1, 1), (2, 16), (3, 32)):
                u = s % 2
                for i in range(5):
                    t0, t1 = kcut[i], kcut[i + 1]
                    gpsimd.dma_start(
                        kt_t[u][0:64, t0:t1], kt[s][:, t0:t1]
                    ).wait_op(pe_v, g + i, GE).then_inc(diK[u], 16)
                for i in range(7):
                    t0, t1 = vcut[i], vcut[i + 1]
                    gpsimd.dma_start(
                        vt_t[u][:, t0:t1], vt[s][:, t0:t1]
                    ).wait_op(pe_v, g + 5 + i, GE).then_inc(diV[u], 16)

        def emit_scores(p, lo=0, hi=9):
            s, hb = divmod(p, PPS)
            b = 2 * hb
            su = s % 2
            if lo == 0 and hb in (0, 8, 9):
                # slot 0: K in 3 pieces (piece1 = scores of pairs 0-7, qt2
                # pairs 8+, lkt2 pairs 9+); slots 1-3: K in 5 even pieces
                # (first 3 cover [0:KP1), 4th covers qt2, 5th lkt2)
                if s == 0:
                    thr = {0: 16, 8: 32, 9: 48}[hb]
                else:
                    base = 48 if s == 2 else (80 if s == 3 else 0)
                    thr = base + {0: 48, 8: 64, 9: 80}[hb]
                nc.tensor.wait_ge(diK[su], thr)
            K = kt_t[su]
            qo = KO_QT1 if b <= 14 else KO_QT2 - 2048
            qA = K[:, qo + b * B : qo + (b + 1) * B]
            qB = K[:, qo + (b + 1) * B : qo + (b + 2) * B]
            qAB = K[:, qo + b * B : qo + (b + 2) * B]
            w1a, w2a = 32 * b, 32 * b + 224
            w1b, w2b = w1a + 32, w2a + 32
            u = p % 2
            lo_l = KO_LKT1 if b <= 16 else KO_LKT2 - 2304
            mms = (
                (C_SP1A, 128, K[:, w1a : w1a + 128], qA),
                (C_SP1B, 128, K[:, w1b : w1b + 128], qB),
                (C_SP2A, 128, K[:, w2a : w2a + 128], qA),
                (C_SP2B, 128, K[:, w2b : w2b + 128], qB),
                (C_G, 256, K[:, KO_GKT : KO_GKT + 128], qAB),
                (C_LOC1, 256, K[:, lo_l + (b + 1) * B : lo_l + (b + 2) * B], qAB),
                (C_LOC0, 128, K[:, lo_l + b * B : lo_l + (b + 1) * B], qA),
                (C_LOC2, 256, K[:, lo_l + (b + 2) * B : lo_l + (b + 3) * B], qAB),
                (C_LOC3, 128, K[:, lo_l + (b + 3) * B : lo_l + (b + 4) * B], qB),
            )
            for kk in range(lo, hi):
                col, w, lhsT, rhs = mms[kk]
                mm = nc.tensor.matmul(
                    psS[u][:, col : col + w],
                    lhsT, rhs,
                    start=True, stop=True,
                )
                if kk == 5 or kk == 8:
                    mm.then_inc(pe_s, 1)

        def pv_mms(p):
            s, hb = divmod(p, PPS)
            b = 2 * hb
            u = p % 2
            su = s % 2
            V = vt_t[su]
            sp = []
            for blk in range(2):
                bb = b + blk
                w1, w2 = 32 * bb, 32 * bb + 224
                c1, r1 = divmod(w1, 128)
                c2, r2 = divmod(w2, 128)
                sp.append((_sv_col(c1, r1 // 32), _sv_col(c2, r2 // 32)))
            lv = [_lv_col(b + k) for k in range(4)]
            outA = pv[u][:, 0:65]
            outB = pv[u][:, 128:193]
            # Sequential accumulation groups (A fully, then B): a start=True
            # marks the surrounding 2KB PSUM zero-region pending-zero, so two
            # interleaved in-flight groups in one bank corrupt each other.
            # (out, pp col, rhs, start, stop)
            return (
                (outA, C_SP1A, V[:, sp[0][0] : sp[0][0] + 65], True, False),
                (outA, C_SP2A, V[:, sp[0][1] : sp[0][1] + 65], False, False),
                (outA, C_G, V[:, 0:65], False, False),
                (outA, C_LOC1, V[:, lv[1] : lv[1] + 65], False, False),
                (outA, C_LOC0, V[:, lv[0] : lv[0] + 65], False, False),
                (outA, C_LOC2, V[:, lv[2] : lv[2] + 65], False, True),
                (outB, C_SP1B, V[:, sp[1][0] : sp[1][0] + 65], True, False),
                (outB, C_SP2B, V[:, sp[1][1] : sp[1][1] + 65], False, False),
                (outB, C_G + 128, V[:, 0:65], False, False),
                (outB, C_LOC1 + 128, V[:, lv[1] : lv[1] + 65], False, False),
                (outB, C_LOC2 + 128, V[:, lv[2] : lv[2] + 65], False, False),
                (outB, C_LOC3, V[:, lv[3] : lv[3] + 65], False, True),
            )

        def emit_pv_range(p, mms, lo, hi):
            u = p % 2
            for kk in range(lo, hi):
                out, col, rhs, st_, sp_ = mms[kk]
                mm = nc.tensor.matmul(
                    out, pp[u][:, col : col + 128], rhs,
                    start=st_, stop=sp_, skip_group_check=True,
                )
                if kk == 11:
                    mm.then_inc(pe_v, 1)

        @block.tensor
        def _(tensor):
            tensor.wait_ge(izv, 1)
            tensor.wait_ge(iza, 1)
            tensor.wait_ge(diK[0], 16)
            emit_scores(0)
            emit_scores(1)
            for p in range(NP):
                s, hb = divmod(p, PPS)
                su = s % 2
                if p >= 2:
                    tensor.wait_ge(dve, p - 1)  # pv[u] free
                if s == 0:
                    if hb == 0:
                        tensor.wait_ge(diP, 16)      # V prefix: pairs 0-3
                    elif hb == 4:
                        tensor.wait_ge(diV[0], 16)   # slot-0 V suffix
                elif hb == 0:
                    # slot 0 V is 1 diV inc (16), slots 1-3 are 7 (112)
                    tensor.wait_ge(diV[su], 16 + 112 * (s // 2) if su == 0
                                   else 112 * (s // 2 + 1))
                mms = pv_mms(p)
                # everything below needs only exp(p) done; scores(p+2) are
                # interleaved so pe_s fires mid-iteration, keeping ACT fed
                tensor.wait_ge(act, p + 1)
                emit_pv_range(p, mms, 0, 4)   # A: sp1 sp2 G loc1
                if p + 2 < NP:
                    emit_scores(p + 2, 0, 6)  # sp x4, G, LOC1
                emit_pv_range(p, mms, 4, 12)  # A: loc0 loc2; B: all
                if p + 2 < NP:
                    emit_scores(p + 2, 6, 9)  # LOC0 LOC2 LOC3

        @block.scalar
        def _(scalar):
            nc.scalar.memzero(kt_t[1][64:128, :]).then_inc(iza, 1)
            # one exp per pair: ACT is the pacer and each ACTIVATE pays a
            # 143ns PSUM-access bubble, so a single op is cheapest.  No pe_v
            # wait needed: pe_s >= 2p+2 means scores(p) mm8 is done, which
            # the in-order PE completed after PV(p-2)'s last matmul, so
            # pp[u] is already free.
            for p in range(NP):
                u = p % 2
                nc.scalar.activation(
                    pp[u][:, 0:1536], psS[u][:, 0:1536], EXP, scale=0.125
                ).wait_op(pe_s, 2 * p + 2, GE).then_inc(act, 1)

        @block.vector
        def _(vector):
            nc.vector.memzero(kt_t[0][64:128, :]).then_inc(izv, 1)
            for p in range(NP):
                u = p % 2
                w = p % OBN
                if p >= OBN:
                    vector.wait_ge(st, 16 * (p - OBN + 1))  # ob[w] stored
                nc.vector.reciprocal(rec[u][:, 0:1], pv[u][:, 64:65]).wait_op(
                    pe_v, p + 1, GE
                )
                nc.vector.reciprocal(rec[u][:, 1:2], pv[u][:, 192:193])
                nc.vector.drain()  # DVE pipeline RAW: rec written, read next
                nc.vector.tensor_mul(
                    ob[w][:, 0:64], pv[u][:, 0:64],
                    rec[u][:, 0:1].broadcast_to([128, 64]),
                )
                nc.vector.tensor_mul(
                    ob[w][:, 64:128], pv[u][:, 128:192],
                    rec[u][:, 1:2].broadcast_to([128, 64]),
                ).then_inc(dve, 1)

        @block.sync
        def _(sync):
            for p in range(NP):
                s, hb = divmod(p, PPS)
                b = 2 * hb
                dst = o[s, b * B : (b + 2) * B, :].rearrange(
                    "(blk q) d -> q blk d", blk=2
                )
                src = ob[p % OBN][:, 0:128].rearrange("q (blk d) -> q blk d", blk=2)
                sync.dma_start(dst, src).wait_op(dve, p + 1, GE).then_inc(st, 16)
            sync.wait_ge(st, 16 * NP)

    return nc


def _prepare(inputs):
    import ml_dtypes

    bf = ml_dtypes.bfloat16
    f = np.float32
    q = np.asarray(inputs["query_layer"], f).reshape(NH, T, D)
    k = np.asarray(inputs["key_layer"], f).reshape(NH, T, D)
    v = np.asarray(inputs["value_layer"], f).reshape(NH, T, D)
    sk = np.asarray(inputs["sparse_key"], f).reshape(NH, TSP, D)
    svv = np.asarray(inputs["sparse_value"], f).reshape(NH, TSP, D)
    gk = np.asarray(inputs["global_key"], f).reshape(NH, G, D)
    gvv = np.asarray(inputs["global_value"], f).reshape(NH, G, D)
    am = np.repeat(np.asarray(inputs["attention_mask"], f)[:, 0, 0, :], H, 0)
    sm = np.repeat(np.asarray(inputs["sparse_mask"], f)[:, 0, 0, :], H, 0)
    gm = np.repeat(np.asarray(inputs["global_mask"], f)[:, 0, 0, :], H, 0)

    # K-side pack [skt | gkt | qt1 | lkt1 | qt2 | lkt2] (see offsets above)
    qt = q.transpose(0, 2, 1)
    lktf = np.zeros((NH, 64, LKT_W), f)
    lktf[:, :, B : B + T] = k.transpose(0, 2, 1)
    kpack = np.zeros((NH, 64, KW), f)
    kpack[:, :, 160 : 160 + TSP] = sk.transpose(0, 2, 1)
    kpack[:, :, KO_GKT : KO_GKT + G] = gk.transpose(0, 2, 1)
    kpack[:, :, KO_QT1 : KO_QT1 + 2048] = qt[:, :, 0:2048]
    kpack[:, :, KO_LKT1 : KO_LKT1 + 2560] = lktf[:, :, 0:2560]
    kpack[:, :, KO_QT2 : KO_QT2 + 2048] = qt[:, :, 2048:4096]
    kpack[:, :, KO_LKT2 : KO_LKT2 + 2048] = lktf[:, :, 2304:4352]
    kpack = kpack.astype(bf)

    # V_aug rows scaled by exp(mask); pad rows are all-zero
    em_l = np.zeros((NH, LKT_W), f)
    em_l[:, B : B + T] = np.exp(am)
    lvp = np.zeros((NH, LKT_W, 65), f)
    lvp[:, B : B + T, :64] = v
    lvp[:, :, 64] = 1.0
    lvp *= em_l[:, :, None]
    lvp = np.ascontiguousarray(
        lvp.reshape(NH, LV_C, 128, 65).transpose(0, 2, 1, 3)
    ).reshape(NH, 128, LV_C * 65)

    SVP_W = 96 + SV_C * 128
    em_s = np.zeros((NH, SVP_W), f)
    em_s[:, 160 : 160 + TSP] = np.exp(sm)
    sv_pad = np.zeros((NH, SVP_W, 65), f)
    sv_pad[:, 160 : 160 + TSP, :64] = svv
    sv_pad[:, :, 64] = 1.0
    sv_pad *= em_s[:, :, None]
    svp = np.empty((NH, 4, 128, SV_C, 65), f)
    for ph in range(4):
        svp[:, ph] = (
            sv_pad[:, 32 * ph : 32 * ph + SV_C * 128]
            .reshape(NH, SV_C, 128, 65)
            .transpose(0, 2, 1, 3)
        )
    # chunk-major [chunk, phase, 65] so the chunks 0-3 prefix is contiguous
    svp = np.ascontiguousarray(svp.transpose(0, 2, 3, 1, 4))  # [NH,128,SV_C,4,65]

    gvp = np.zeros((NH, 128, 65), f)
    gvp[:, :G, :64] = gvv
    gvp[:, :G, 64] = 1.0
    gvp[:, :G] *= np.exp(gm)[:, :, None]

    vpack = np.concatenate(
        [
            gvp,
            svp[:, :, :SV_PRE].reshape(NH, 128, SV_PRE * 4 * 65),
            lvp[:, :, : LV_PRE * 65],
            svp[:, :, SV_PRE:].reshape(NH, 128, (SV_C - SV_PRE) * 4 * 65),
            lvp[:, :, LV_PRE * 65 :],
        ],
        axis=2,
    ).astype(bf)

    return [
        {
            "kt": kpack[c * SL : (c + 1) * SL],
            "vt": vpack[c * SL : (c + 1) * SL],
        }
        for c in range(NCORES)
    ]


_NC_CACHE = {}
LAST_RESULTS = None


def kernel(**inputs):
    global LAST_RESULTS
    if "nc" not in _NC_CACHE:
        _NC_CACHE["nc"] = _build_bass()
    nc = _NC_CACHE["nc"]
    in_maps = _prepare(inputs)
    res = run_bass_kernel_spmd(nc, in_maps, core_ids=list(range(NCORES)))
    LAST_RESULTS = res
    out = np.empty((NH, T, D), np.float32)
    for c in range(NCORES):
        out[c * SL : (c + 1) * SL] = res.results[c]["o"]
    return out.reshape(N, H, T, D)
